# revision 5
# baseline (speedup 1.0000x reference)
"""Causal self-attention on 8 trn2 NeuronCores — v2.

Sharding: core c = (b, g) with b = c // 4 (batch), g = c % 4 (head group of
4 heads).  Each core computes q/k/v projections for its 4 heads, causal
attention, and a partial out-projection (its 256 rows of Wout).  Host sums
the 4 partials per batch and adds bout.

v2 structural changes vs v1:
  * q/k/v projections run as fp8e4m3 DoubleRow matmuls with hi+lo error
    compensation (W' = 32*W split into Whi+Wlo, x into xhi+xlo; the three
    products Whi.xhi + Whi.xlo + Wlo.xhi land in one f32 psum).  25% fewer
    PE cycles than bf16 at ~bf16 accuracy; the 32x scale folds into the
    exp scale (q,k) and the v evacuation (x 1/32).
  * attnV swaps moving/stationary: expt tiles [128k x 128q] are the
    stationary operand, vaug [128k x 65] the moving one, accumulating into
    per-head psum accumulators [q, 4qt, 65] — 65-cycle matmuls instead of
    width-cycle ones (2x fewer PE cycles), with the softmax denominator in
    column 64 via the vaug ones-column.
  * normalization fuses into the accumulator evacuation (tensor_tensor with
    a stride-0-broadcast reciprocal), then PE transposes [q, f] -> [f, q]
    tiles through identity is_transpose matmuls for the out-projection.
  * psum accumulation uses one start=True per 2KB bank zero-region; sibling
    chains open start=False and rely on pending-zero (all psum tags are
    bank-sized so regions never straddle tiles).

Layouts on device:
  xhi/xlo  [128, 8, 2048] fp8   x[b]^T, d-tile major
  wh*/wl*  [128, 8, 256]  fp8   32*W columns for this group, d-tile major
  qT/kT    [128, 2, 2048] bf16  [2 heads x 64 hd][pair][pos], carries x32
  vaug     [128, 16, 4, 65] bf16  per k-tile, per head: 64 v-cols + ones
  expt     [128, 2, 512]  bf16  exp(scores^T) per k-tile, [k][head][q]
  attnT    [128, 2, 2048] bf16  normalized attn, features on partitions
"""

import sys

if "/opt/trn_rl_repo" not in sys.path:
    sys.path.insert(0, "/opt/trn_rl_repo")

import numpy as np

import concourse.mybir as mybir
import concourse.tile as tile
from concourse import bacc
from concourse.bass_utils import run_bass_kernel_spmd
from concourse.vector_clock import ScopedClock, VectorClock

B, S, D, H, HD = 2, 2048, 1024, 16, 64
G = 4            # head groups (cores per batch)
HL = H // G      # heads per core = 4
FL = HL * HD     # local features = 256
NQB = S // 512   # 4 q-blocks of 512
NST = S // 128   # 16 s-tiles of 128
NDT = D // 128   # 8 d-tiles
NDP = NDT // 2   # 4 d-tile pairs for DoubleRow

F32 = mybir.dt.float32
BF16 = mybir.dt.bfloat16
F8 = mybir.dt.float8e4
EXPF = mybir.ActivationFunctionType.Exp
DR = mybir.MatmulPerfMode.DoubleRow
MUL = mybir.AluOpType.mult

WSCALE = 32.0                    # W' = 32*W for fp8 hi/lo headroom
SCEXP = 0.125 / (WSCALE * WSCALE)  # exp scale: 1/sqrt(HD) / (32*32)


class SplitDrainTC(tile.TileContext):
    """This walrus build rejects >1 sync wait on an SP Drain; emit one
    drain per live proc instead of a single fat one."""

    def _drain_and_barrier(self, tick_clock, wait_clock):
        g = tick_clock.global_clock
        n = len(g)
        live = [(p, g[p]) for p in range(n) if g[p] > 0]
        if not live:
            self.nc.sync.drain()
        for p, t in live:
            vec = [0] * n
            vec[p] = t
            d = self.nc.sync.drain()
            wait_clock.add_sem_waits(d.ins, ScopedClock({None: VectorClock(vec)}))
        self.nc.all_engine_barrier()
        assert self.sems is not None
        popped = self.nc._tile_sem_poison_stack.pop()
        assert popped is self._sem_poison
        self.nc.clear_and_free_semaphores(list(self.sems.allocated().values()))
        self.nc.all_engine_barrier()


def _build(debug=False):
    nc = bacc.Bacc()
    xhi = nc.declare_dram_parameter("xhi", [128, NDT, S], F8, isOutput=False)
    xlo = nc.declare_dram_parameter("xlo", [128, NDT, S], F8, isOutput=False)
    whq = nc.declare_dram_parameter("whq", [128, NDT, FL], F8, isOutput=False)
    wlq = nc.declare_dram_parameter("wlq", [128, NDT, FL], F8, isOutput=False)
    whk = nc.declare_dram_parameter("whk", [128, NDT, FL], F8, isOutput=False)
    wlk = nc.declare_dram_parameter("wlk", [128, NDT, FL], F8, isOutput=False)
    whv = nc.declare_dram_parameter("whv", [128, NDT, FL], F8, isOutput=False)
    wlv = nc.declare_dram_parameter("wlv", [128, NDT, FL], F8, isOutput=False)
    wout = nc.declare_dram_parameter("wout", [128, 2, D], BF16, isOutput=False)
    tri2 = nc.declare_dram_parameter("tri2", [128, 2, 128], BF16, isOutput=False)
    ident = nc.declare_dram_parameter("ident", [128, 128], BF16, isOutput=False)
    out_p = nc.declare_dram_parameter("out_p", [S, D], BF16, isOutput=True)

    from collections import deque
    from contextlib import ExitStack

    with SplitDrainTC(nc) as tc, ExitStack() as ctx:
        consts = ctx.enter_context(tc.tile_pool(name="consts", bufs=1))
        pp_fill = ctx.enter_context(tc.tile_pool(name="pp_fill", bufs=2, space="PSUM"))
        attn_ctx = ExitStack()
        pp_sc = attn_ctx.enter_context(tc.tile_pool(name="pp_sc", bufs=2, space="PSUM"))
        pp_acc = attn_ctx.enter_context(
            tc.tile_pool(name="pp_acc", bufs=1, space="PSUM")
        )
        oproj_pool = [pp_fill]
        pool_exp = ctx.enter_context(tc.tile_pool(name="pool_exp", bufs=7))
        pool_out = ctx.enter_context(tc.tile_pool(name="pool_out", bufs=5))
        pool_sm = ctx.enter_context(tc.tile_pool(name="pool_sm", bufs=4))

        xhi_sb = consts.tile([128, NDT, S], F8)
        xlo_sb = consts.tile([128, NDT, S], F8)
        whq_sb = consts.tile([128, NDT, FL], F8)
        wlq_sb = consts.tile([128, NDT, FL], F8)
        whk_sb = consts.tile([128, NDT, FL], F8)
        wlk_sb = consts.tile([128, NDT, FL], F8)
        whv_sb = consts.tile([128, NDT, FL], F8)
        wlv_sb = consts.tile([128, NDT, FL], F8)
        wout_sb = consts.tile([128, 2, D], BF16)
        tri2_sb = consts.tile([128, 2, 128], BF16)
        id_sb = consts.tile([128, 128], BF16)
        qT_sb = consts.tile([128, 2, S], BF16)
        kT_sb = consts.tile([128, 2, S], BF16)
        vaug_sb = consts.tile([128, NST, HL, HD + 1], BF16)
        attnT_sb = consts.tile([128, 2, S], BF16)

        # PE clock-ramp warmup: dummy matmuls on zeroed SBUF while the
        # first DMAs land, so real matmuls start at full clock.
        nc.vector.memset(attnT_sb[:, 0, 0:256], 0.0)
        for i in range(64):
            wps = pp_fill.tile([128, 512], F32, tag="fill")
            nc.tensor.matmul(
                wps[:, 0:128],
                attnT_sb[:, 0, 0:128],
                attnT_sb[:, 0, 128:256],
                start=True,
                stop=True,
            )

        # DMA order matters: first matmuls need wq hi/lo and the first
        # s-block of xhi/xlo; weights issue from the (idle-at-start) ACT
        # queue so their descriptor generation runs parallel to the x
        # stream on SP.
        nc.scalar.dma_start(out=whq_sb, in_=whq[:])
        nc.scalar.dma_start(out=wlq_sb, in_=wlq[:])
        nc.sync.dma_start(out=xhi_sb[:, :, 0:512], in_=xhi[:, :, 0:512])
        nc.scalar.dma_start(out=whk_sb, in_=whk[:])
        nc.scalar.dma_start(out=wlk_sb, in_=wlk[:])
        nc.scalar.dma_start(out=tri2_sb, in_=tri2[:])
        nc.sync.dma_start(out=xlo_sb[:, :, 0:512], in_=xlo[:, :, 0:512])
        nc.scalar.dma_start(out=whv_sb, in_=whv[:])
        nc.scalar.dma_start(out=wlv_sb, in_=wlv[:])
        nc.scalar.dma_start(out=id_sb, in_=ident[:])
        nc.sync.dma_start(out=xhi_sb[:, :, 512:1024], in_=xhi[:, :, 512:1024])
        nc.sync.dma_start(out=xlo_sb[:, :, 512:1024], in_=xlo[:, :, 512:1024])
        nc.scalar.dma_start(out=wout_sb, in_=wout[:])
        nc.sync.dma_start(out=xhi_sb[:, :, 1024:S], in_=xhi[:, :, 1024:S])
        nc.sync.dma_start(out=xlo_sb[:, :, 1024:S], in_=xlo[:, :, 1024:S])
        # ACT spline-table preload for Exp, after the weight DMA issues so
        # it doesn't delay them on the ACT queue
        warm = pool_sm.tile([1, 1], F32, tag="warm")
        nc.vector.memset(warm, 0.0)
        nc.scalar.activation(out=warm, in_=warm, func=EXPF)
        # ones columns of vaug (constant across the run)
        nc.gpsimd.memset(vaug_sb[:, :, :, HD : HD + 1], 1.0)

        # ---- chunk emitters (projections / out-proj used as PE filler) ----
        def qkT_chunk(wh_sb, wl_sb, dst, ft, sb_):
            # 512 positions of one 128-feature column tile of q or k:
            # 2 pos-chunks x (4 d-pairs x 3 comp terms) DoubleRow matmuls.
            def emit():
                ps = pp_fill.tile([128, 512], F32, tag="fill")
                for c in range(2):
                    p0 = sb_ * 512 + c * 256
                    first = True
                    for dp in range(NDP):
                        dsl = slice(2 * dp, 2 * dp + 2)
                        fsl = slice(ft * 128, ft * 128 + 128)
                        for wmat, xmat in (
                            (wh_sb, xhi_sb),
                            (wl_sb, xhi_sb),
                            (wh_sb, xlo_sb),
                        ):
                            nc.tensor.matmul(
                                ps[:, c * 256 : c * 256 + 256],
                                wmat[:, dsl, fsl],
                                xmat[:, dsl, p0 : p0 + 256],
                                start=first,
                                stop=(dp == NDP - 1 and xmat is xlo_sb),
                                perf_mode=DR,
                                skip_group_check=True,
                            )
                            first = False
                nc.vector.tensor_copy(
                    out=dst[:, ft, sb_ * 512 : sb_ * 512 + 512], in_=ps[:, 0:512]
                )

            return emit

        def v_chunk(st):
            def emit():
                ps = pp_fill.tile([128, 512], F32, tag="fill")
                first = True
                for dp in range(NDP):
                    dsl = slice(2 * dp, 2 * dp + 2)
                    psl = slice(st * 128, st * 128 + 128)
                    for xmat, wmat in (
                        (xhi_sb, whv_sb),
                        (xlo_sb, whv_sb),
                        (xhi_sb, wlv_sb),
                    ):
                        nc.tensor.matmul(
                            ps[:, 0:FL],
                            xmat[:, dsl, psl],
                            wmat[:, dsl, :],
                            start=first,
                            stop=(dp == NDP - 1 and wmat is wlv_sb),
                            perf_mode=DR,
                            skip_group_check=True,
                        )
                        first = False
                # evacuate with the 1/32 descale (W' = 32*W)
                nc.vector.tensor_scalar(
                    out=vaug_sb[:, st, :, 0:HD],
                    in0=ps[:, 0:FL].rearrange("p (h e) -> p h e", h=HL),
                    scalar1=1.0 / WSCALE,
                    scalar2=None,
                    op0=MUL,
                )

            return emit

        def oproj_tail(q0):
            # tail variant: both 512-col halves of a q-tile, one combined
            # 2KB DMA; DVE and ACT each evacuate one half
            def emit():
                out_t = pool_out.tile([128, 2, 512], BF16, tag="outw")
                for dc in range(2):
                    ops = oproj_pool[0].tile([128, 512], F32, tag="fill")
                    for ft in range(2):
                        nc.tensor.matmul(
                            ops[:, 0:512],
                            attnT_sb[:, ft, q0 : q0 + 128],
                            wout_sb[:, ft, dc * 512 : dc * 512 + 512],
                            start=(ft == 0),
                            stop=(ft == 1),
                        )
                    if dc == 0:
                        nc.vector.tensor_copy(out=out_t[:, 0, :], in_=ops[:, 0:512])
                    else:
                        nc.scalar.copy(out=out_t[:, 1, :], in_=ops[:, 0:512])
                nc.sync.dma_start(
                    out=out_p[q0 : q0 + 128, :],
                    in_=out_t.rearrange("p a b -> p (a b)"),
                )

            return emit

        def oproj_half(q0, dc, late=False):
            def emit():
                ops = oproj_pool[0].tile([128, 512], F32, tag="fill")
                for ft in range(2):
                    nc.tensor.matmul(
                        ops[:, 0:512],
                        attnT_sb[:, ft, q0 : q0 + 128],
                        wout_sb[:, ft, dc * 512 : dc * 512 + 512],
                        start=(ft == 0),
                        stop=(ft == 1),
                    )
                out_t = pool_out.tile([128, 512], BF16, tag="out")
                if late and dc == 1:
                    # post-attention: ACT is idle, split the evacuations
                    nc.scalar.copy(out=out_t, in_=ops[:, 0:512])
                else:
                    nc.vector.tensor_copy(out=out_t, in_=ops[:, 0:512])
                nc.sync.dma_start(
                    out=out_p[q0 : q0 + 128, dc * 512 : dc * 512 + 512], in_=out_t
                )

            return emit

        # filler queue: (deadline, cost_ns, emit_fn); FIFO order respects deps.
        # deadline units: 2*qb + pair (+0.5 for "before this pair's attnV
        # drain"); drain_due forces everything due at each boundary.
        queue = deque()
        reserve = deque()
        for qb in range(NQB):
            for wh_sb, wl_sb, dst in (
                (whq_sb, wlq_sb, qT_sb),
                (whk_sb, wlk_sb, kT_sb),
            ):
                if qb > 0:
                    queue.append(
                        (2 * qb - 1.25, 1300, qkT_chunk(wh_sb, wl_sb, dst, 0, qb))
                    )
            for st in range(4 * qb, 4 * qb + 4):
                queue.append((2 * qb + 0.5, 650, v_chunk(st)))
            for wh_sb, wl_sb, dst in (
                (whq_sb, wlq_sb, qT_sb),
                (whk_sb, wlk_sb, kT_sb),
            ):
                queue.append(
                    (
                        max(0.75, 2 * qb - 0.25),
                        1300,
                        qkT_chunk(wh_sb, wl_sb, dst, 1, qb),
                    )
                )

        # Adaptive pump: spread remaining filler cost over remaining attention
        # steps so late q-blocks (which have no projections left) still get
        # out-proj chunks as PE filler.
        total_steps = sum(2 * (4 * qb + 4) for qb in range(NQB))  # 80
        future_oproj = 4 * NQB * 900
        step_no = 0

        tokens = 0.0
        PUMP_RATE = 340.0  # ~per-step PE deficit vs the ACT exp stream

        def pump():
            nonlocal step_no, future_oproj, tokens
            step_no += 1
            tokens += PUMP_RATE
            while queue and tokens >= queue[0][1]:
                _, cost, emit = queue.popleft()
                emit()
                tokens -= cost

        def drain_due(qb):
            while queue and queue[0][0] <= qb:
                _, _, emit = queue.popleft()
                emit()

        # ---- prologue: only what (qb0, pair0) scores need; the rest
        # streams in as filler during pair0 ----
        qkT_chunk(whq_sb, wlq_sb, qT_sb, 0, 0)()
        qkT_chunk(whk_sb, wlk_sb, kT_sb, 0, 0)()

        # deferred per-(qb,pair) epilogue (transposes + attnT evac), emitted
        # a few kb-steps into the NEXT pair so PE never waits on the DVE
        # normalize chain
        epi_q = deque()
        norm_q = deque()
        staged = deque()

        def epilogue_tail(attnq, pair_, qb_):
            def emit():
                tp = pp_fill.tile([128, 512], F32, tag="fill")
                tpb = tp.bitcast(BF16)
                for qt in range(4):
                    nc.tensor.matmul(
                        tpb[:, qt * 128 : qt * 128 + 128],
                        attnq[:, qt, :, :].rearrange("p h f -> p (h f)"),
                        id_sb,
                        start=(qt == 0),
                        stop=(qt == 3),
                        is_transpose=True,
                        skip_group_check=True,
                    )
                nc.vector.tensor_copy(
                    out=attnT_sb[:, pair_, qb_ * 512 : qb_ * 512 + 512],
                    in_=tpb[:, 0:512],
                )
                if pair_ == 1:
                    # attnT for qb_ is complete -> its out-proj becomes
                    # filler, but hold it a few kb-steps so the pump can't
                    # pop it while the attnT evacuation is still in flight.
                    for qs_ in range(4):
                        for dc_ in range(2):
                            staged.append(
                                (
                                    100,
                                    450,
                                    oproj_half(
                                        qb_ * 512 + qs_ * 128,
                                        dc_,
                                        late=(qb_ == NQB - 1),
                                    ),
                                )
                            )

            return emit

        # ---- attention (scores -> exp/mask -> lagged swapped attnV) ----
        for qb in range(NQB):
            for pair in range(2):
                drain_due(2 * qb + pair)
                nkb = 4 * qb + 4
                # acc tiles are allocated lazily at kb==2, after the previous
                # pair's deferred normalize has been emitted (pool WAR
                # tracking needs readers emitted before the next allocation)
                accv = [None, None]

                def alloc_acc(accv=accv):
                    for h in range(2):
                        a = pp_acc.tile([128, 512], F32, tag=f"acc{h}")
                        accv[h] = a[:, 0:260].rearrange("p (a c) -> p a c", c=HD + 1)

                lagged = deque()  # expt tiles awaiting their attnV matmuls

                def attnv(expt, kb, r, accv=accv, pair=pair, qb=qb):
                    for qt in range(max(r, 0), 4):
                        for h in range(2):
                            nc.tensor.matmul(
                                accv[h][:, qt, :],
                                expt[:, h, qt * 128 : qt * 128 + 128],
                                vaug_sb[:, kb, 2 * pair + h, :],
                                start=(kb == 0 and qt == max(r, 0)),
                                stop=(kb == 4 * qb + qt),
                                skip_group_check=True,
                            )

                for kb in range(nkb):
                    r = kb - 4 * qb
                    soff = 128 * max(r, 0)
                    sps = pp_sc.tile([128, 2, 512], F32, tag="ps")
                    for h in range(2):
                        hp = slice(64 * h, 64 * h + 64)
                        nc.tensor.matmul(
                            sps[:, h, soff:512],
                            kT_sb[hp, pair, kb * 128 : kb * 128 + 128],
                            qT_sb[hp, pair, qb * 512 + soff : qb * 512 + 512],
                            start=True,
                            stop=True,
                        )
                    expt = pool_exp.tile([128, 2, 512], BF16, tag="expt")
                    if r <= 0:
                        nc.scalar.activation(
                            out=expt.rearrange("p h q -> p (h q)"),
                            in_=sps.rearrange("p h q -> p (h q)"),
                            func=EXPF,
                            scale=SCEXP,
                        )
                    else:
                        nc.scalar.activation(
                            out=expt[:, :, soff:512],
                            in_=sps[:, :, soff:512],
                            func=EXPF,
                            scale=SCEXP,
                        )
                    if r >= 0:
                        # within-tile causal mask on the diagonal strip; the
                        # last diagonals gate the pair-end attnV drain, so
                        # run them on DVE (no Q7 launch latency)
                        tri_eng = nc.vector if r >= 2 else nc.gpsimd
                        tri_eng.tensor_tensor(
                            out=expt[:, :, soff : soff + 128],
                            in0=expt[:, :, soff : soff + 128],
                            in1=tri2_sb,
                            op=MUL,
                        )
                    lagged.append((expt, kb, r))
                    if len(lagged) > 5:
                        attnv(*lagged.popleft())
                    if kb == 1 and norm_q:
                        norm_q.popleft()()
                    if kb == 2:
                        alloc_acc()
                    if epi_q and kb == 3:
                        epi_q.popleft()()
                    if kb >= 6 and staged:
                        queue.extend(staged)
                        staged.clear()
                    pump()
                queue.extend(staged)
                staged.clear()
                drain_due(2 * qb + pair + 0.5)
                last = qb == NQB - 1 and pair == 1
                if last:
                    # h-major drain: head 0 finishes first so its normalize
                    # overlaps head 1's remaining matmuls
                    tail_kbs = list(lagged)
                    lagged.clear()
                else:
                    while lagged:
                        attnv(*lagged.popleft())
                while epi_q:
                    epi_q.popleft()()

                # normalize off the accumulators: batched reciprocal of the
                # ones-column denominators, then fused mult-evacuate to bf16.
                # Deferred into the next pair's kb==1 so the DVE chain never
                # sits at the PE queue head during the pair transition.
                attnq = pool_sm.tile([128, 4, 2, HD], BF16, tag="attnq")
                rec = pool_sm.tile([128, 2, 4], F32, tag="rec")

                def norm_h(h, accv=accv, attnq=attnq, rec=rec):
                    nc.vector.reciprocal(
                        out=rec[:, h, :],
                        in_=accv[h][:, :, HD : HD + 1].rearrange("p a c -> p (a c)"),
                    )
                    nc.vector.tensor_tensor(
                        out=attnq[:, :, h, :],
                        in0=accv[h][:, :, 0:HD],
                        in1=rec[:, h, :].broadcast_to([128, 4, HD]),
                        op=MUL,
                    )

                def norm_emit():
                    norm_h(0)
                    norm_h(1)

                ep = epilogue_tail(attnq, pair, qb)
                if last:
                    for h in range(2):
                        for expt_, kb_, r_ in tail_kbs:
                            for qt in range(max(r_, 0), 4):
                                nc.tensor.matmul(
                                    accv[h][:, qt, :],
                                    expt_[:, h, qt * 128 : qt * 128 + 128],
                                    vaug_sb[:, kb_, 2 * pair + h, :],
                                    start=False,
                                    stop=(kb_ == 4 * qb + qt),
                                    skip_group_check=True,
                                )
                        norm_h(h)
                    ep()
                else:
                    norm_q.append(norm_emit)
                    epi_q.append(ep)

            if qb == NQB - 1:
                for qs in range(4):
                    reserve.append(oproj_tail(qb * 512 + qs * 128))
            future_oproj -= 4 * 900

        attn_ctx.close()
        pp_tail = ctx.enter_context(
            tc.tile_pool(name="pp_tail", bufs=4, space="PSUM")
        )
        oproj_pool[0] = pp_tail
        while reserve:
            reserve.popleft()()
        while queue:
            _, _, emit = queue.popleft()
            emit()

    nc.compile()
    return nc


_NC = None


def _get_nc():
    global _NC
    if _NC is None:
        _NC = _build()
    return _NC


def kernel(x, mask, Wqkv, bqkv, Wout, bout):
    x = np.asarray(x, dtype=np.float32)
    Wqkv = np.asarray(Wqkv, dtype=np.float32)
    bqkv = np.asarray(bqkv, dtype=np.float32)
    Wout = np.asarray(Wout, dtype=np.float32)
    bout = np.asarray(bout, dtype=np.float32)
    assert not np.any(bqkv), "nonzero bqkv not supported by this kernel"

    import ml_dtypes

    bf16 = ml_dtypes.bfloat16
    f8 = ml_dtypes.float8_e4m3

    def hilo(a):
        hi = a.astype(f8)
        lo = (a - hi.astype(np.float32)).astype(f8)
        return np.ascontiguousarray(hi), np.ascontiguousarray(lo)

    # host-side layout prep; x and the qkv weights ship as fp8 hi/lo pairs
    xhis, xlos = [], []
    for b in range(B):
        xt = x[b].T.reshape(NDT, 128, S).transpose(1, 0, 2)  # [128, 8, 2048]
        hi, lo = hilo(xt)
        xhis.append(hi)
        xlos.append(lo)
    tri = np.triu(np.ones((128, 128), dtype=np.float32)).astype(bf16)
    tri2 = np.ascontiguousarray(np.stack([tri, tri], axis=1))  # [128, 2, 128]
    identv = np.ascontiguousarray(np.eye(128, dtype=np.float32).astype(bf16))

    def wslice(j, g):  # j: 0=q,1=k,2=v -> hi/lo [128, 8, 256] fp8
        cols = Wqkv[:, j * D + g * FL : j * D + (g + 1) * FL] * WSCALE
        wt = cols.reshape(NDT, 128, FL).transpose(1, 0, 2)
        return hilo(wt)

    in_maps = []
    for c in range(8):
        b, g = c // G, c % G
        whq_, wlq_ = wslice(0, g)
        whk_, wlk_ = wslice(1, g)
        whv_, wlv_ = wslice(2, g)
        wo = Wout[g * FL : (g + 1) * FL, :]  # [256, 1024]
        in_maps.append(
            {
                "xhi": xhis[b],
                "xlo": xlos[b],
                "whq": whq_,
                "wlq": wlq_,
                "whk": whk_,
                "wlk": wlk_,
                "whv": whv_,
                "wlv": wlv_,
                "wout": np.ascontiguousarray(
                    wo.reshape(2, 128, D).transpose(1, 0, 2).astype(bf16)
                ),
                "tri2": tri2,
                "ident": identv,
            }
        )

    nc = _get_nc()
    # axon terminals occasionally flake (transient NRT_EXEC_UNIT errors);
    # a retry of the same dispatch succeeds
    import time as _time

    res = None
    for attempt in range(3):
        try:
            res = run_bass_kernel_spmd(nc, in_maps, core_ids=list(range(8)))
            break
        except Exception:
            if attempt == 2:
                raise
            _time.sleep(2.0)

    out = np.empty((B, S, D), dtype=np.float32)
    for b in range(B):
        acc = res.results[b * G]["out_p"].astype(np.float32).copy()
        for g in range(1, G):
            acc += res.results[b * G + g]["out_p"]
        out[b] = acc + bout[None, :]
    return out


# revision 7
# speedup vs baseline: 1.0021x; 1.0021x over previous
"""Causal self-attention on 8 trn2 NeuronCores — v2.

Sharding: core c = (b, g) with b = c // 4 (batch), g = c % 4 (head group of
4 heads).  Each core computes q/k/v projections for its 4 heads, causal
attention, and a partial out-projection (its 256 rows of Wout).  Host sums
the 4 partials per batch and adds bout.

v2 structural changes vs v1:
  * q/k/v projections run as fp8e4m3 DoubleRow matmuls with hi+lo error
    compensation (W' = 32*W split into Whi+Wlo, x into xhi+xlo; the three
    products Whi.xhi + Whi.xlo + Wlo.xhi land in one f32 psum).  25% fewer
    PE cycles than bf16 at ~bf16 accuracy; the 32x scale folds into the
    exp scale (q,k) and the v evacuation (x 1/32).
  * attnV swaps moving/stationary: expt tiles [128k x 128q] are the
    stationary operand, vaug [128k x 65] the moving one, accumulating into
    per-head psum accumulators [q, 4qt, 65] — 65-cycle matmuls instead of
    width-cycle ones (2x fewer PE cycles), with the softmax denominator in
    column 64 via the vaug ones-column.
  * normalization fuses into the accumulator evacuation (tensor_tensor with
    a stride-0-broadcast reciprocal), then PE transposes [q, f] -> [f, q]
    tiles through identity is_transpose matmuls for the out-projection.
  * psum accumulation uses one start=True per 2KB bank zero-region; sibling
    chains open start=False and rely on pending-zero (all psum tags are
    bank-sized so regions never straddle tiles).

Layouts on device:
  xhi/xlo  [128, 8, 2048] fp8   x[b]^T, d-tile major
  wh*/wl*  [128, 8, 256]  fp8   32*W columns for this group, d-tile major
  qT/kT    [128, 2, 2048] bf16  [2 heads x 64 hd][pair][pos], carries x32
  vaug     [128, 16, 4, 65] bf16  per k-tile, per head: 64 v-cols + ones
  expt     [128, 2, 512]  bf16  exp(scores^T) per k-tile, [k][head][q]
  attnT    [128, 2, 2048] bf16  normalized attn, features on partitions
"""

import sys

if "/opt/trn_rl_repo" not in sys.path:
    sys.path.insert(0, "/opt/trn_rl_repo")

import numpy as np

import concourse.mybir as mybir
import concourse.tile as tile
from concourse import bacc
from concourse.bass_utils import run_bass_kernel_spmd
from concourse.vector_clock import ScopedClock, VectorClock

B, S, D, H, HD = 2, 2048, 1024, 16, 64
G = 4            # head groups (cores per batch)
HL = H // G      # heads per core = 4
FL = HL * HD     # local features = 256
NQB = S // 512   # 4 q-blocks of 512
NST = S // 128   # 16 s-tiles of 128
NDT = D // 128   # 8 d-tiles
NDP = NDT // 2   # 4 d-tile pairs for DoubleRow

F32 = mybir.dt.float32
BF16 = mybir.dt.bfloat16
F8 = mybir.dt.float8e4
EXPF = mybir.ActivationFunctionType.Exp
DR = mybir.MatmulPerfMode.DoubleRow
MUL = mybir.AluOpType.mult

WSCALE = 32.0                    # W' = 32*W for fp8 hi/lo headroom
SCEXP = 0.125 / (WSCALE * WSCALE)  # exp scale: 1/sqrt(HD) / (32*32)


class SplitDrainTC(tile.TileContext):
    """This walrus build rejects >1 sync wait on an SP Drain; emit one
    drain per live proc instead of a single fat one."""

    def _drain_and_barrier(self, tick_clock, wait_clock):
        g = tick_clock.global_clock
        n = len(g)
        live = [(p, g[p]) for p in range(n) if g[p] > 0]
        if not live:
            self.nc.sync.drain()
        for p, t in live:
            vec = [0] * n
            vec[p] = t
            d = self.nc.sync.drain()
            wait_clock.add_sem_waits(d.ins, ScopedClock({None: VectorClock(vec)}))
        self.nc.all_engine_barrier()
        assert self.sems is not None
        popped = self.nc._tile_sem_poison_stack.pop()
        assert popped is self._sem_poison
        self.nc.clear_and_free_semaphores(list(self.sems.allocated().values()))
        self.nc.all_engine_barrier()


def _build(debug=False):
    nc = bacc.Bacc()
    xhi = nc.declare_dram_parameter("xhi", [128, NDT, S], F8, isOutput=False)
    xlo = nc.declare_dram_parameter("xlo", [128, NDT, S], F8, isOutput=False)
    whq = nc.declare_dram_parameter("whq", [128, NDT, FL], F8, isOutput=False)
    wlq = nc.declare_dram_parameter("wlq", [128, NDT, FL], F8, isOutput=False)
    whk = nc.declare_dram_parameter("whk", [128, NDT, FL], F8, isOutput=False)
    wlk = nc.declare_dram_parameter("wlk", [128, NDT, FL], F8, isOutput=False)
    whv = nc.declare_dram_parameter("whv", [128, NDT, FL], F8, isOutput=False)
    wlv = nc.declare_dram_parameter("wlv", [128, NDT, FL], F8, isOutput=False)
    wout = nc.declare_dram_parameter("wout", [128, 2, D], BF16, isOutput=False)
    tri2 = nc.declare_dram_parameter("tri2", [128, 2, 128], BF16, isOutput=False)
    ident = nc.declare_dram_parameter("ident", [128, 128], BF16, isOutput=False)
    out_p = nc.declare_dram_parameter("out_p", [S, D], BF16, isOutput=True)

    from collections import deque
    from contextlib import ExitStack

    with SplitDrainTC(nc) as tc, ExitStack() as ctx:
        consts = ctx.enter_context(tc.tile_pool(name="consts", bufs=1))
        pp_fill = ctx.enter_context(tc.tile_pool(name="pp_fill", bufs=2, space="PSUM"))
        attn_ctx = ExitStack()
        pp_sc = attn_ctx.enter_context(tc.tile_pool(name="pp_sc", bufs=2, space="PSUM"))
        pp_acc = attn_ctx.enter_context(
            tc.tile_pool(name="pp_acc", bufs=1, space="PSUM")
        )
        oproj_pool = [pp_fill]
        pool_exp = ctx.enter_context(tc.tile_pool(name="pool_exp", bufs=7))
        pool_out = ctx.enter_context(tc.tile_pool(name="pool_out", bufs=5))
        pool_sm = ctx.enter_context(tc.tile_pool(name="pool_sm", bufs=4))

        xhi_sb = consts.tile([128, NDT, S], F8)
        xlo_sb = consts.tile([128, NDT, S], F8)
        whq_sb = consts.tile([128, NDT, FL], F8)
        wlq_sb = consts.tile([128, NDT, FL], F8)
        whk_sb = consts.tile([128, NDT, FL], F8)
        wlk_sb = consts.tile([128, NDT, FL], F8)
        whv_sb = consts.tile([128, NDT, FL], F8)
        wlv_sb = consts.tile([128, NDT, FL], F8)
        wout_sb = consts.tile([128, 2, D], BF16)
        tri2_sb = consts.tile([128, 2, 128], BF16)
        id_sb = consts.tile([128, 128], BF16)
        qT_sb = consts.tile([128, 2, S], BF16)
        kT_sb = consts.tile([128, 2, S], BF16)
        vaug_sb = consts.tile([128, NST, HL, HD + 1], BF16)
        attnT_sb = consts.tile([128, 2, S], BF16)

        # PE clock-ramp warmup: dummy matmuls on zeroed SBUF while the
        # first DMAs land, so real matmuls start at full clock.
        nc.vector.memset(attnT_sb[:, 0, 0:256], 0.0)
        for i in range(64):
            wps = pp_fill.tile([128, 512], F32, tag="fill")
            nc.tensor.matmul(
                wps[:, 0:128],
                attnT_sb[:, 0, 0:128],
                attnT_sb[:, 0, 128:256],
                start=True,
                stop=True,
            )

        # DMA order matters: first matmuls need wq hi/lo and the first
        # s-block of xhi/xlo; weights issue from the (idle-at-start) ACT
        # queue so their descriptor generation runs parallel to the x
        # stream on SP.
        nc.scalar.dma_start(out=whq_sb, in_=whq[:])
        nc.scalar.dma_start(out=wlq_sb, in_=wlq[:])
        nc.sync.dma_start(out=xhi_sb[:, :, 0:512], in_=xhi[:, :, 0:512])
        nc.scalar.dma_start(out=whk_sb, in_=whk[:])
        nc.scalar.dma_start(out=wlk_sb, in_=wlk[:])
        nc.scalar.dma_start(out=tri2_sb, in_=tri2[:])
        nc.sync.dma_start(out=xlo_sb[:, :, 0:512], in_=xlo[:, :, 0:512])
        nc.scalar.dma_start(out=whv_sb, in_=whv[:])
        nc.scalar.dma_start(out=wlv_sb, in_=wlv[:])
        nc.scalar.dma_start(out=id_sb, in_=ident[:])
        nc.sync.dma_start(out=xhi_sb[:, :, 512:1024], in_=xhi[:, :, 512:1024])
        nc.sync.dma_start(out=xlo_sb[:, :, 512:1024], in_=xlo[:, :, 512:1024])
        nc.scalar.dma_start(out=wout_sb, in_=wout[:])
        nc.sync.dma_start(out=xhi_sb[:, :, 1024:S], in_=xhi[:, :, 1024:S])
        nc.sync.dma_start(out=xlo_sb[:, :, 1024:S], in_=xlo[:, :, 1024:S])
        # ACT spline-table preload for Exp, after the weight DMA issues so
        # it doesn't delay them on the ACT queue
        warm = pool_sm.tile([1, 1], F32, tag="warm")
        nc.vector.memset(warm, 0.0)
        nc.scalar.activation(out=warm, in_=warm, func=EXPF)
        # ones columns of vaug (constant across the run)
        nc.gpsimd.memset(vaug_sb[:, :, :, HD : HD + 1], 1.0)

        # ---- chunk emitters (projections / out-proj used as PE filler) ----
        def qkT_chunk(wh_sb, wl_sb, dst, ft, sb_):
            # 512 positions of one 128-feature column tile of q or k:
            # 2 pos-chunks x (4 d-pairs x 3 comp terms) DoubleRow matmuls.
            def emit():
                ps = pp_fill.tile([128, 512], F32, tag="fill")
                for c in range(2):
                    p0 = sb_ * 512 + c * 256
                    first = True
                    for dp in range(NDP):
                        dsl = slice(2 * dp, 2 * dp + 2)
                        fsl = slice(ft * 128, ft * 128 + 128)
                        for wmat, xmat in (
                            (wh_sb, xhi_sb),
                            (wl_sb, xhi_sb),
                            (wh_sb, xlo_sb),
                        ):
                            nc.tensor.matmul(
                                ps[:, c * 256 : c * 256 + 256],
                                wmat[:, dsl, fsl],
                                xmat[:, dsl, p0 : p0 + 256],
                                start=first,
                                stop=(dp == NDP - 1 and xmat is xlo_sb),
                                perf_mode=DR,
                                skip_group_check=True,
                            )
                            first = False
                nc.vector.tensor_copy(
                    out=dst[:, ft, sb_ * 512 : sb_ * 512 + 512], in_=ps[:, 0:512]
                )

            return emit

        def v_chunk(st):
            def emit():
                ps = pp_fill.tile([128, 512], F32, tag="fill")
                first = True
                for dp in range(NDP):
                    dsl = slice(2 * dp, 2 * dp + 2)
                    psl = slice(st * 128, st * 128 + 128)
                    for xmat, wmat in (
                        (xhi_sb, whv_sb),
                        (xlo_sb, whv_sb),
                        (xhi_sb, wlv_sb),
                    ):
                        nc.tensor.matmul(
                            ps[:, 0:FL],
                            xmat[:, dsl, psl],
                            wmat[:, dsl, :],
                            start=first,
                            stop=(dp == NDP - 1 and wmat is wlv_sb),
                            perf_mode=DR,
                            skip_group_check=True,
                        )
                        first = False
                # evacuate with the 1/32 descale (W' = 32*W)
                nc.vector.tensor_scalar(
                    out=vaug_sb[:, st, :, 0:HD],
                    in0=ps[:, 0:FL].rearrange("p (h e) -> p h e", h=HL),
                    scalar1=1.0 / WSCALE,
                    scalar2=None,
                    op0=MUL,
                )

            return emit

        def oproj_tail(q0):
            # tail variant: both 512-col halves of a q-tile, one combined
            # 2KB DMA; DVE and ACT each evacuate one half
            def emit():
                out_t = pool_out.tile([128, 2, 512], BF16, tag="outw")
                for dc in range(2):
                    ops = oproj_pool[0].tile([128, 512], F32, tag="fill")
                    for ft in range(2):
                        nc.tensor.matmul(
                            ops[:, 0:512],
                            attnT_sb[:, ft, q0 : q0 + 128],
                            wout_sb[:, ft, dc * 512 : dc * 512 + 512],
                            start=(ft == 0),
                            stop=(ft == 1),
                        )
                    if dc == 0:
                        nc.vector.tensor_copy(out=out_t[:, 0, :], in_=ops[:, 0:512])
                    else:
                        nc.scalar.copy(out=out_t[:, 1, :], in_=ops[:, 0:512])
                nc.sync.dma_start(
                    out=out_p[q0 : q0 + 128, :],
                    in_=out_t.rearrange("p a b -> p (a b)"),
                )

            return emit

        def oproj_half(q0, dc, late=False):
            def emit():
                ops = oproj_pool[0].tile([128, 512], F32, tag="fill")
                for ft in range(2):
                    nc.tensor.matmul(
                        ops[:, 0:512],
                        attnT_sb[:, ft, q0 : q0 + 128],
                        wout_sb[:, ft, dc * 512 : dc * 512 + 512],
                        start=(ft == 0),
                        stop=(ft == 1),
                    )
                out_t = pool_out.tile([128, 512], BF16, tag="out")
                if late and dc == 1:
                    # post-attention: ACT is idle, split the evacuations
                    nc.scalar.copy(out=out_t, in_=ops[:, 0:512])
                else:
                    nc.vector.tensor_copy(out=out_t, in_=ops[:, 0:512])
                nc.sync.dma_start(
                    out=out_p[q0 : q0 + 128, dc * 512 : dc * 512 + 512], in_=out_t
                )

            return emit

        # filler queue: (deadline, cost_ns, emit_fn); FIFO order respects deps.
        # deadline units: 2*qb + pair (+0.5 for "before this pair's attnV
        # drain"); drain_due forces everything due at each boundary.
        queue = deque()
        reserve = deque()
        for qb in range(NQB):
            for wh_sb, wl_sb, dst in (
                (whq_sb, wlq_sb, qT_sb),
                (whk_sb, wlk_sb, kT_sb),
            ):
                if qb > 0:
                    queue.append(
                        (2 * qb - 1.25, 1300, qkT_chunk(wh_sb, wl_sb, dst, 0, qb))
                    )
            for st in range(4 * qb, 4 * qb + 4):
                queue.append((2 * qb + 0.5, 650, v_chunk(st)))
            for wh_sb, wl_sb, dst in (
                (whq_sb, wlq_sb, qT_sb),
                (whk_sb, wlk_sb, kT_sb),
            ):
                queue.append(
                    (
                        max(0.75, 2 * qb - 0.25),
                        1300,
                        qkT_chunk(wh_sb, wl_sb, dst, 1, qb),
                    )
                )

        # Adaptive pump: spread remaining filler cost over remaining attention
        # steps so late q-blocks (which have no projections left) still get
        # out-proj chunks as PE filler.
        total_steps = sum(2 * (4 * qb + 4) for qb in range(NQB))  # 80
        future_oproj = 4 * NQB * 900
        step_no = 0

        tokens = 0.0
        PUMP_RATE = 355.0  # ~per-step PE deficit vs the ACT exp stream

        def pump():
            nonlocal step_no, future_oproj, tokens
            step_no += 1
            tokens += PUMP_RATE
            while queue and tokens >= queue[0][1]:
                _, cost, emit = queue.popleft()
                emit()
                tokens -= cost

        def drain_due(qb):
            while queue and queue[0][0] <= qb:
                _, _, emit = queue.popleft()
                emit()

        # ---- prologue: only what (qb0, pair0) scores need; the rest
        # streams in as filler during pair0 ----
        qkT_chunk(whq_sb, wlq_sb, qT_sb, 0, 0)()
        qkT_chunk(whk_sb, wlk_sb, kT_sb, 0, 0)()

        # deferred per-(qb,pair) epilogue (transposes + attnT evac), emitted
        # a few kb-steps into the NEXT pair so PE never waits on the DVE
        # normalize chain
        epi_q = deque()
        norm_q = deque()
        staged = deque()

        def epilogue_tail(attnq, pair_, qb_):
            def emit():
                tp = pp_fill.tile([128, 512], F32, tag="fill")
                tpb = tp.bitcast(BF16)
                for qt in range(4):
                    nc.tensor.matmul(
                        tpb[:, qt * 128 : qt * 128 + 128],
                        attnq[:, qt, :, :].rearrange("p h f -> p (h f)"),
                        id_sb,
                        start=(qt == 0),
                        stop=(qt == 3),
                        is_transpose=True,
                        skip_group_check=True,
                    )
                nc.vector.tensor_copy(
                    out=attnT_sb[:, pair_, qb_ * 512 : qb_ * 512 + 512],
                    in_=tpb[:, 0:512],
                )
                if pair_ == 1:
                    # attnT for qb_ is complete -> its out-proj becomes
                    # filler, but hold it a few kb-steps so the pump can't
                    # pop it while the attnT evacuation is still in flight.
                    for qs_ in range(4):
                        for dc_ in range(2):
                            staged.append(
                                (
                                    100,
                                    450,
                                    oproj_half(
                                        qb_ * 512 + qs_ * 128,
                                        dc_,
                                        late=(qb_ == NQB - 1),
                                    ),
                                )
                            )

            return emit

        # ---- attention (scores -> exp/mask -> lagged swapped attnV) ----
        for qb in range(NQB):
            for pair in range(2):
                drain_due(2 * qb + pair)
                nkb = 4 * qb + 4
                # acc tiles are allocated lazily at kb==2, after the previous
                # pair's deferred normalize has been emitted (pool WAR
                # tracking needs readers emitted before the next allocation)
                accv = [None, None]

                def alloc_acc(accv=accv):
                    for h in range(2):
                        a = pp_acc.tile([128, 512], F32, tag=f"acc{h}")
                        accv[h] = a[:, 0:260].rearrange("p (a c) -> p a c", c=HD + 1)

                lagged = deque()  # expt tiles awaiting their attnV matmuls

                def attnv(expt, kb, r, accv=accv, pair=pair, qb=qb):
                    for qt in range(max(r, 0), 4):
                        for h in range(2):
                            nc.tensor.matmul(
                                accv[h][:, qt, :],
                                expt[:, h, qt * 128 : qt * 128 + 128],
                                vaug_sb[:, kb, 2 * pair + h, :],
                                start=(kb == 0 and qt == max(r, 0)),
                                stop=(kb == 4 * qb + qt),
                                skip_group_check=True,
                            )

                for kb in range(nkb):
                    r = kb - 4 * qb
                    soff = 128 * max(r, 0)
                    sps = pp_sc.tile([128, 2, 512], F32, tag="ps")
                    for h in range(2):
                        hp = slice(64 * h, 64 * h + 64)
                        nc.tensor.matmul(
                            sps[:, h, soff:512],
                            kT_sb[hp, pair, kb * 128 : kb * 128 + 128],
                            qT_sb[hp, pair, qb * 512 + soff : qb * 512 + 512],
                            start=True,
                            stop=True,
                        )
                    expt = pool_exp.tile([128, 2, 512], BF16, tag="expt")
                    if r <= 0:
                        nc.scalar.activation(
                            out=expt.rearrange("p h q -> p (h q)"),
                            in_=sps.rearrange("p h q -> p (h q)"),
                            func=EXPF,
                            scale=SCEXP,
                        )
                    else:
                        nc.scalar.activation(
                            out=expt[:, :, soff:512],
                            in_=sps[:, :, soff:512],
                            func=EXPF,
                            scale=SCEXP,
                        )
                    if r >= 0:
                        # within-tile causal mask on the diagonal strip; the
                        # last diagonals gate the pair-end attnV drain, so
                        # run them on DVE (no Q7 launch latency)
                        tri_eng = nc.vector if r >= 2 else nc.gpsimd
                        tri_eng.tensor_tensor(
                            out=expt[:, :, soff : soff + 128],
                            in0=expt[:, :, soff : soff + 128],
                            in1=tri2_sb,
                            op=MUL,
                        )
                    lagged.append((expt, kb, r))
                    if len(lagged) > 5:
                        attnv(*lagged.popleft())
                    if kb == 1 and norm_q:
                        norm_q.popleft()()
                    if kb == 2:
                        alloc_acc()
                    if epi_q and kb == 3:
                        epi_q.popleft()()
                    if kb >= 6 and staged:
                        queue.extend(staged)
                        staged.clear()
                    pump()
                queue.extend(staged)
                staged.clear()
                drain_due(2 * qb + pair + 0.5)
                last = qb == NQB - 1 and pair == 1
                if last:
                    # h-major drain: head 0 finishes first so its normalize
                    # overlaps head 1's remaining matmuls
                    tail_kbs = list(lagged)
                    lagged.clear()
                else:
                    while lagged:
                        attnv(*lagged.popleft())
                while epi_q:
                    epi_q.popleft()()

                # normalize off the accumulators: batched reciprocal of the
                # ones-column denominators, then fused mult-evacuate to bf16.
                # Deferred into the next pair's kb==1 so the DVE chain never
                # sits at the PE queue head during the pair transition.
                attnq = pool_sm.tile([128, 4, 2, HD], BF16, tag="attnq")
                rec = pool_sm.tile([128, 2, 4], F32, tag="rec")

                def norm_h(h, accv=accv, attnq=attnq, rec=rec):
                    nc.vector.reciprocal(
                        out=rec[:, h, :],
                        in_=accv[h][:, :, HD : HD + 1].rearrange("p a c -> p (a c)"),
                    )
                    nc.vector.tensor_tensor(
                        out=attnq[:, :, h, :],
                        in0=accv[h][:, :, 0:HD],
                        in1=rec[:, h, :].broadcast_to([128, 4, HD]),
                        op=MUL,
                    )

                def norm_emit():
                    norm_h(0)
                    norm_h(1)

                ep = epilogue_tail(attnq, pair, qb)
                if last:
                    for h in range(2):
                        for expt_, kb_, r_ in tail_kbs:
                            for qt in range(max(r_, 0), 4):
                                nc.tensor.matmul(
                                    accv[h][:, qt, :],
                                    expt_[:, h, qt * 128 : qt * 128 + 128],
                                    vaug_sb[:, kb_, 2 * pair + h, :],
                                    start=False,
                                    stop=(kb_ == 4 * qb + qt),
                                    skip_group_check=True,
                                )
                        norm_h(h)
                    ep()
                else:
                    norm_q.append(norm_emit)
                    epi_q.append(ep)

            if qb == NQB - 1:
                for qs in range(4):
                    reserve.append(oproj_tail(qb * 512 + qs * 128))
            future_oproj -= 4 * 900

        attn_ctx.close()
        pp_tail = ctx.enter_context(
            tc.tile_pool(name="pp_tail", bufs=4, space="PSUM")
        )
        oproj_pool[0] = pp_tail
        while reserve:
            reserve.popleft()()
        while queue:
            _, _, emit = queue.popleft()
            emit()

    nc.compile()
    return nc


_NC = None


def _get_nc():
    global _NC
    if _NC is None:
        _NC = _build()
    return _NC


def kernel(x, mask, Wqkv, bqkv, Wout, bout):
    x = np.asarray(x, dtype=np.float32)
    Wqkv = np.asarray(Wqkv, dtype=np.float32)
    bqkv = np.asarray(bqkv, dtype=np.float32)
    Wout = np.asarray(Wout, dtype=np.float32)
    bout = np.asarray(bout, dtype=np.float32)
    assert not np.any(bqkv), "nonzero bqkv not supported by this kernel"

    import ml_dtypes

    bf16 = ml_dtypes.bfloat16
    f8 = ml_dtypes.float8_e4m3

    def hilo(a):
        hi = a.astype(f8)
        lo = (a - hi.astype(np.float32)).astype(f8)
        return np.ascontiguousarray(hi), np.ascontiguousarray(lo)

    # host-side layout prep; x and the qkv weights ship as fp8 hi/lo pairs
    xhis, xlos = [], []
    for b in range(B):
        xt = x[b].T.reshape(NDT, 128, S).transpose(1, 0, 2)  # [128, 8, 2048]
        hi, lo = hilo(xt)
        xhis.append(hi)
        xlos.append(lo)
    tri = np.triu(np.ones((128, 128), dtype=np.float32)).astype(bf16)
    tri2 = np.ascontiguousarray(np.stack([tri, tri], axis=1))  # [128, 2, 128]
    identv = np.ascontiguousarray(np.eye(128, dtype=np.float32).astype(bf16))

    def wslice(j, g):  # j: 0=q,1=k,2=v -> hi/lo [128, 8, 256] fp8
        cols = Wqkv[:, j * D + g * FL : j * D + (g + 1) * FL] * WSCALE
        wt = cols.reshape(NDT, 128, FL).transpose(1, 0, 2)
        return hilo(wt)

    in_maps = []
    for c in range(8):
        b, g = c // G, c % G
        whq_, wlq_ = wslice(0, g)
        whk_, wlk_ = wslice(1, g)
        whv_, wlv_ = wslice(2, g)
        wo = Wout[g * FL : (g + 1) * FL, :]  # [256, 1024]
        in_maps.append(
            {
                "xhi": xhis[b],
                "xlo": xlos[b],
                "whq": whq_,
                "wlq": wlq_,
                "whk": whk_,
                "wlk": wlk_,
                "whv": whv_,
                "wlv": wlv_,
                "wout": np.ascontiguousarray(
                    wo.reshape(2, 128, D).transpose(1, 0, 2).astype(bf16)
                ),
                "tri2": tri2,
                "ident": identv,
            }
        )

    nc = _get_nc()
    # axon terminals occasionally flake: transient NRT_EXEC_UNIT errors
    # (caught+retried) but also rare silent numeric corruption on a core.
    # Dispatch twice and cross-check; on mismatch, a third run breaks the
    # tie (device execution is deterministic, so good runs agree exactly).
    import time as _time

    def dispatch():
        for attempt in range(3):
            try:
                res = run_bass_kernel_spmd(nc, in_maps, core_ids=list(range(8)))
                break
            except Exception:
                if attempt == 2:
                    raise
                _time.sleep(2.0)
        out = np.empty((B, S, D), dtype=np.float32)
        for b in range(B):
            acc = res.results[b * G]["out_p"].astype(np.float32).copy()
            for g in range(1, G):
                acc += res.results[b * G + g]["out_p"]
            out[b] = acc + bout[None, :]
        return out

    def close(a, b):
        return np.linalg.norm(a - b) <= 1e-4 * np.linalg.norm(a)

    out1 = dispatch()
    out2 = dispatch()
    if close(out1, out2):
        return out1
    out3 = dispatch()
    if close(out1, out3):
        return out1
    if close(out2, out3):
        return out2
    return out3


# revision 8
# speedup vs baseline: 1.0128x; 1.0107x over previous
"""Causal self-attention on 8 trn2 NeuronCores — v2.

Sharding: core c = (b, g) with b = c // 4 (batch), g = c % 4 (head group of
4 heads).  Each core computes q/k/v projections for its 4 heads, causal
attention, and a partial out-projection (its 256 rows of Wout).  Host sums
the 4 partials per batch and adds bout.

v2 structural changes vs v1:
  * q/k/v projections run as fp8e4m3 DoubleRow matmuls with hi+lo error
    compensation (W' = 32*W split into Whi+Wlo, x into xhi+xlo; the three
    products Whi.xhi + Whi.xlo + Wlo.xhi land in one f32 psum).  25% fewer
    PE cycles than bf16 at ~bf16 accuracy; the 32x scale folds into the
    exp scale (q,k) and the v evacuation (x 1/32).
  * attnV swaps moving/stationary: expt tiles [128k x 128q] are the
    stationary operand, vaug [128k x 65] the moving one, accumulating into
    per-head psum accumulators [q, 4qt, 65] — 65-cycle matmuls instead of
    width-cycle ones (2x fewer PE cycles), with the softmax denominator in
    column 64 via the vaug ones-column.
  * normalization fuses into the accumulator evacuation (tensor_tensor with
    a stride-0-broadcast reciprocal), then PE transposes [q, f] -> [f, q]
    tiles through identity is_transpose matmuls for the out-projection.
  * psum accumulation uses one start=True per 2KB bank zero-region; sibling
    chains open start=False and rely on pending-zero (all psum tags are
    bank-sized so regions never straddle tiles).

Layouts on device:
  xhi/xlo  [128, 8, 2048] fp8   x[b]^T, d-tile major
  wh*/wl*  [128, 8, 256]  fp8   32*W columns for this group, d-tile major
  qT/kT    [128, 2, 2048] bf16  [2 heads x 64 hd][pair][pos], carries x32
  vaug     [128, 16, 4, 65] bf16  per k-tile, per head: 64 v-cols + ones
  expt     [128, 2, 512]  bf16  exp(scores^T) per k-tile, [k][head][q]
  attnT    [128, 2, 2048] bf16  normalized attn, features on partitions
"""

import sys

if "/opt/trn_rl_repo" not in sys.path:
    sys.path.insert(0, "/opt/trn_rl_repo")

import numpy as np

import concourse.mybir as mybir
import concourse.tile as tile
from concourse import bacc
from concourse.bass_utils import run_bass_kernel_spmd
from concourse.vector_clock import ScopedClock, VectorClock

B, S, D, H, HD = 2, 2048, 1024, 16, 64
G = 4            # head groups (cores per batch)
HL = H // G      # heads per core = 4
FL = HL * HD     # local features = 256
NQB = S // 512   # 4 q-blocks of 512
NST = S // 128   # 16 s-tiles of 128
NDT = D // 128   # 8 d-tiles
NDP = NDT // 2   # 4 d-tile pairs for DoubleRow

F32 = mybir.dt.float32
BF16 = mybir.dt.bfloat16
F8 = mybir.dt.float8e4
EXPF = mybir.ActivationFunctionType.Exp
DR = mybir.MatmulPerfMode.DoubleRow
MUL = mybir.AluOpType.mult

WSCALE = 32.0                    # W' = 32*W for fp8 hi/lo headroom
SCEXP = 0.125 / (WSCALE * WSCALE)  # exp scale: 1/sqrt(HD) / (32*32)


class SplitDrainTC(tile.TileContext):
    """This walrus build rejects >1 sync wait on an SP Drain; emit one
    drain per live proc instead of a single fat one."""

    def _drain_and_barrier(self, tick_clock, wait_clock):
        g = tick_clock.global_clock
        n = len(g)
        live = [(p, g[p]) for p in range(n) if g[p] > 0]
        if not live:
            self.nc.sync.drain()
        for p, t in live:
            vec = [0] * n
            vec[p] = t
            d = self.nc.sync.drain()
            wait_clock.add_sem_waits(d.ins, ScopedClock({None: VectorClock(vec)}))
        self.nc.all_engine_barrier()
        assert self.sems is not None
        popped = self.nc._tile_sem_poison_stack.pop()
        assert popped is self._sem_poison
        self.nc.clear_and_free_semaphores(list(self.sems.allocated().values()))
        self.nc.all_engine_barrier()


def _build(debug=False):
    nc = bacc.Bacc()
    xhi = nc.declare_dram_parameter("xhi", [128, NDT, S], F8, isOutput=False)
    xlo = nc.declare_dram_parameter("xlo", [128, NDT, S], F8, isOutput=False)
    whq = nc.declare_dram_parameter("whq", [128, NDT, FL], F8, isOutput=False)
    wlq = nc.declare_dram_parameter("wlq", [128, NDT, FL], F8, isOutput=False)
    whk = nc.declare_dram_parameter("whk", [128, NDT, FL], F8, isOutput=False)
    wlk = nc.declare_dram_parameter("wlk", [128, NDT, FL], F8, isOutput=False)
    whv = nc.declare_dram_parameter("whv", [128, NDT, FL], F8, isOutput=False)
    wlv = nc.declare_dram_parameter("wlv", [128, NDT, FL], F8, isOutput=False)
    wout = nc.declare_dram_parameter("wout", [128, 2, D], BF16, isOutput=False)
    tri2 = nc.declare_dram_parameter("tri2", [128, 2, 128], BF16, isOutput=False)
    ident = nc.declare_dram_parameter("ident", [128, 128], BF16, isOutput=False)
    out_p = nc.declare_dram_parameter("out_p", [S, D], BF16, isOutput=True)

    from collections import deque
    from contextlib import ExitStack

    with SplitDrainTC(nc) as tc, ExitStack() as ctx:
        consts = ctx.enter_context(tc.tile_pool(name="consts", bufs=1))
        pp_fill = ctx.enter_context(tc.tile_pool(name="pp_fill", bufs=2, space="PSUM"))
        attn_ctx = ExitStack()
        pp_sc = attn_ctx.enter_context(tc.tile_pool(name="pp_sc", bufs=2, space="PSUM"))
        pp_acc = attn_ctx.enter_context(
            tc.tile_pool(name="pp_acc", bufs=1, space="PSUM")
        )
        oproj_pool = [pp_fill]
        pool_exp = ctx.enter_context(tc.tile_pool(name="pool_exp", bufs=7))
        pool_out = ctx.enter_context(tc.tile_pool(name="pool_out", bufs=5))
        pool_sm = ctx.enter_context(tc.tile_pool(name="pool_sm", bufs=4))

        xhi_sb = consts.tile([128, NDT, S], F8)
        xlo_sb = consts.tile([128, NDT, S], F8)
        whq_sb = consts.tile([128, NDT, FL], F8)
        wlq_sb = consts.tile([128, NDT, FL], F8)
        whk_sb = consts.tile([128, NDT, FL], F8)
        wlk_sb = consts.tile([128, NDT, FL], F8)
        whv_sb = consts.tile([128, NDT, FL], F8)
        wlv_sb = consts.tile([128, NDT, FL], F8)
        wout_sb = consts.tile([128, 2, D], BF16)
        tri2_sb = consts.tile([128, 2, 128], BF16)
        id_sb = consts.tile([128, 128], BF16)
        qT_sb = consts.tile([128, 2, S], BF16)
        kT_sb = consts.tile([128, 2, S], BF16)
        vaug_sb = consts.tile([128, NST, HL, HD + 1], BF16)
        attnT_sb = consts.tile([128, 2, S], BF16)

        # PE clock-ramp warmup: dummy matmuls on zeroed SBUF while the
        # first DMAs land, so real matmuls start at full clock.
        nc.vector.memset(attnT_sb[:, 0, 0:256], 0.0)
        for i in range(64):
            wps = pp_fill.tile([128, 512], F32, tag="fill")
            nc.tensor.matmul(
                wps[:, 0:128],
                attnT_sb[:, 0, 0:128],
                attnT_sb[:, 0, 128:256],
                start=True,
                stop=True,
            )

        # DMA order matters: first matmuls need wq hi/lo and the first
        # s-block of xhi/xlo; weights issue from the (idle-at-start) ACT
        # queue so their descriptor generation runs parallel to the x
        # stream on SP.
        nc.scalar.dma_start(out=whq_sb, in_=whq[:])
        nc.scalar.dma_start(out=wlq_sb, in_=wlq[:])
        nc.sync.dma_start(out=xhi_sb[:, :, 0:512], in_=xhi[:, :, 0:512])
        nc.scalar.dma_start(out=whk_sb, in_=whk[:])
        nc.scalar.dma_start(out=wlk_sb, in_=wlk[:])
        nc.scalar.dma_start(out=tri2_sb, in_=tri2[:])
        nc.sync.dma_start(out=xlo_sb[:, :, 0:512], in_=xlo[:, :, 0:512])
        nc.scalar.dma_start(out=whv_sb, in_=whv[:])
        nc.scalar.dma_start(out=wlv_sb, in_=wlv[:])
        nc.scalar.dma_start(out=id_sb, in_=ident[:])
        nc.sync.dma_start(out=xhi_sb[:, :, 512:1024], in_=xhi[:, :, 512:1024])
        nc.sync.dma_start(out=xlo_sb[:, :, 512:1024], in_=xlo[:, :, 512:1024])
        nc.scalar.dma_start(out=wout_sb, in_=wout[:])
        nc.sync.dma_start(out=xhi_sb[:, :, 1024:S], in_=xhi[:, :, 1024:S])
        nc.sync.dma_start(out=xlo_sb[:, :, 1024:S], in_=xlo[:, :, 1024:S])
        # ACT spline-table preload for Exp, after the weight DMA issues so
        # it doesn't delay them on the ACT queue
        warm = pool_sm.tile([1, 1], F32, tag="warm")
        nc.vector.memset(warm, 0.0)
        nc.scalar.activation(out=warm, in_=warm, func=EXPF)
        # ones columns of vaug (constant across the run)
        nc.gpsimd.memset(vaug_sb[:, :, :, HD : HD + 1], 1.0)

        # ---- chunk emitters (projections / out-proj used as PE filler) ----
        def qkT_chunk(wh_sb, wl_sb, dst, ft, sb_):
            # 512 positions of one 128-feature column tile of q or k:
            # 2 pos-chunks x (4 d-pairs x 3 comp terms) DoubleRow matmuls.
            def emit():
                ps = pp_fill.tile([128, 512], F32, tag="fill")
                for c in range(2):
                    p0 = sb_ * 512 + c * 256
                    first = True
                    for dp in range(NDP):
                        dsl = slice(2 * dp, 2 * dp + 2)
                        fsl = slice(ft * 128, ft * 128 + 128)
                        for wmat, xmat in (
                            (wh_sb, xhi_sb),
                            (wl_sb, xhi_sb),
                            (wh_sb, xlo_sb),
                        ):
                            nc.tensor.matmul(
                                ps[:, c * 256 : c * 256 + 256],
                                wmat[:, dsl, fsl],
                                xmat[:, dsl, p0 : p0 + 256],
                                start=first,
                                stop=(dp == NDP - 1 and xmat is xlo_sb),
                                perf_mode=DR,
                                skip_group_check=True,
                            )
                            first = False
                nc.vector.tensor_copy(
                    out=dst[:, ft, sb_ * 512 : sb_ * 512 + 512], in_=ps[:, 0:512]
                )

            return emit

        def v_chunk(st):
            def emit():
                ps = pp_fill.tile([128, 512], F32, tag="fill")
                first = True
                for dp in range(NDP):
                    dsl = slice(2 * dp, 2 * dp + 2)
                    psl = slice(st * 128, st * 128 + 128)
                    for xmat, wmat in (
                        (xhi_sb, whv_sb),
                        (xlo_sb, whv_sb),
                        (xhi_sb, wlv_sb),
                    ):
                        nc.tensor.matmul(
                            ps[:, 0:FL],
                            xmat[:, dsl, psl],
                            wmat[:, dsl, :],
                            start=first,
                            stop=(dp == NDP - 1 and wmat is wlv_sb),
                            perf_mode=DR,
                            skip_group_check=True,
                        )
                        first = False
                # evacuate with the 1/32 descale (W' = 32*W)
                nc.vector.tensor_scalar(
                    out=vaug_sb[:, st, :, 0:HD],
                    in0=ps[:, 0:FL].rearrange("p (h e) -> p h e", h=HL),
                    scalar1=1.0 / WSCALE,
                    scalar2=None,
                    op0=MUL,
                )

            return emit

        def oproj_tail(q0):
            # tail variant: both 512-col halves of a q-tile, one combined
            # 2KB DMA; DVE and ACT each evacuate one half
            def emit():
                out_t = pool_out.tile([128, 2, 512], BF16, tag="outw")
                for dc in range(2):
                    ops = oproj_pool[0].tile([128, 512], F32, tag="fill")
                    for ft in range(2):
                        nc.tensor.matmul(
                            ops[:, 0:512],
                            attnT_sb[:, ft, q0 : q0 + 128],
                            wout_sb[:, ft, dc * 512 : dc * 512 + 512],
                            start=(ft == 0),
                            stop=(ft == 1),
                        )
                    if dc == 0:
                        nc.vector.tensor_copy(out=out_t[:, 0, :], in_=ops[:, 0:512])
                    else:
                        nc.scalar.copy(out=out_t[:, 1, :], in_=ops[:, 0:512])
                nc.sync.dma_start(
                    out=out_p[q0 : q0 + 128, :],
                    in_=out_t.rearrange("p a b -> p (a b)"),
                )

            return emit

        def oproj_half(q0, dc, late=False):
            def emit():
                ops = oproj_pool[0].tile([128, 512], F32, tag="fill")
                for ft in range(2):
                    nc.tensor.matmul(
                        ops[:, 0:512],
                        attnT_sb[:, ft, q0 : q0 + 128],
                        wout_sb[:, ft, dc * 512 : dc * 512 + 512],
                        start=(ft == 0),
                        stop=(ft == 1),
                    )
                out_t = pool_out.tile([128, 512], BF16, tag="out")
                if late and dc == 1:
                    # post-attention: ACT is idle, split the evacuations
                    nc.scalar.copy(out=out_t, in_=ops[:, 0:512])
                else:
                    nc.vector.tensor_copy(out=out_t, in_=ops[:, 0:512])
                nc.sync.dma_start(
                    out=out_p[q0 : q0 + 128, dc * 512 : dc * 512 + 512], in_=out_t
                )

            return emit

        # filler queue: (deadline, cost_ns, emit_fn); FIFO order respects deps.
        # deadline units: 2*qb + pair (+0.5 for "before this pair's attnV
        # drain"); drain_due forces everything due at each boundary.
        queue = deque()
        reserve = deque()
        for qb in range(NQB):
            for wh_sb, wl_sb, dst in (
                (whq_sb, wlq_sb, qT_sb),
                (whk_sb, wlk_sb, kT_sb),
            ):
                if qb > 0:
                    queue.append(
                        (2 * qb - 1.25, 1300, qkT_chunk(wh_sb, wl_sb, dst, 0, qb))
                    )
            for st in range(4 * qb, 4 * qb + 4):
                queue.append((2 * qb + 0.5, 650, v_chunk(st)))
            for wh_sb, wl_sb, dst in (
                (whq_sb, wlq_sb, qT_sb),
                (whk_sb, wlk_sb, kT_sb),
            ):
                queue.append(
                    (
                        max(0.75, 2 * qb - 0.25),
                        1300,
                        qkT_chunk(wh_sb, wl_sb, dst, 1, qb),
                    )
                )

        # Adaptive pump: spread remaining filler cost over remaining attention
        # steps so late q-blocks (which have no projections left) still get
        # out-proj chunks as PE filler.
        total_steps = sum(2 * (4 * qb + 4) for qb in range(NQB))  # 80
        future_oproj = 4 * NQB * 900
        step_no = 0

        tokens = 0.0
        PUMP_RATE = 355.0  # ~per-step PE deficit vs the ACT exp stream

        def pump():
            nonlocal step_no, future_oproj, tokens
            step_no += 1
            tokens += PUMP_RATE
            while queue and tokens >= queue[0][1]:
                _, cost, emit = queue.popleft()
                emit()
                tokens -= cost

        def drain_due(qb):
            while queue and queue[0][0] <= qb:
                _, _, emit = queue.popleft()
                emit()

        # ---- prologue: only what (qb0, pair0) scores need; the rest
        # streams in as filler during pair0 ----
        qkT_chunk(whq_sb, wlq_sb, qT_sb, 0, 0)()
        qkT_chunk(whk_sb, wlk_sb, kT_sb, 0, 0)()

        # deferred per-(qb,pair) epilogue (transposes + attnT evac), emitted
        # a few kb-steps into the NEXT pair so PE never waits on the DVE
        # normalize chain
        epi_q = deque()
        norm_q = deque()
        staged = deque()

        def epilogue_tail(attnq, pair_, qb_):
            def emit():
                tp = pp_fill.tile([128, 512], F32, tag="fill")
                tpb = tp.bitcast(BF16)
                for qt in range(4):
                    nc.tensor.matmul(
                        tpb[:, qt * 128 : qt * 128 + 128],
                        attnq[:, qt, :, :].rearrange("p h f -> p (h f)"),
                        id_sb,
                        start=(qt == 0),
                        stop=(qt == 3),
                        is_transpose=True,
                        skip_group_check=True,
                    )
                nc.vector.tensor_copy(
                    out=attnT_sb[:, pair_, qb_ * 512 : qb_ * 512 + 512],
                    in_=tpb[:, 0:512],
                )
                if pair_ == 1:
                    # attnT for qb_ is complete -> its out-proj becomes
                    # filler, but hold it a few kb-steps so the pump can't
                    # pop it while the attnT evacuation is still in flight.
                    for qs_ in range(4):
                        for dc_ in range(2):
                            staged.append(
                                (
                                    100,
                                    450,
                                    oproj_half(
                                        qb_ * 512 + qs_ * 128,
                                        dc_,
                                        late=(qb_ == NQB - 1),
                                    ),
                                )
                            )

            return emit

        # ---- attention (scores -> exp/mask -> lagged swapped attnV) ----
        for qb in range(NQB):
            for pair in range(2):
                drain_due(2 * qb + pair)
                nkb = 4 * qb + 4
                # acc tiles are allocated lazily at kb==2, after the previous
                # pair's deferred normalize has been emitted (pool WAR
                # tracking needs readers emitted before the next allocation)
                accv = [None, None]

                def alloc_acc(accv=accv):
                    for h in range(2):
                        a = pp_acc.tile([128, 512], F32, tag=f"acc{h}")
                        accv[h] = a[:, 0:260].rearrange("p (a c) -> p a c", c=HD + 1)

                lagged = deque()  # expt tiles awaiting their attnV matmuls

                def attnv(expt, kb, r, accv=accv, pair=pair, qb=qb):
                    for qt in range(max(r, 0), 4):
                        for h in range(2):
                            nc.tensor.matmul(
                                accv[h][:, qt, :],
                                expt[:, h, qt * 128 : qt * 128 + 128],
                                vaug_sb[:, kb, 2 * pair + h, :],
                                start=(kb == 0 and qt == max(r, 0)),
                                stop=(kb == 4 * qb + qt),
                                skip_group_check=True,
                            )

                for kb in range(nkb):
                    r = kb - 4 * qb
                    soff = 128 * max(r, 0)
                    sps = pp_sc.tile([128, 2, 512], F32, tag="ps")
                    for h in range(2):
                        hp = slice(64 * h, 64 * h + 64)
                        nc.tensor.matmul(
                            sps[:, h, soff:512],
                            kT_sb[hp, pair, kb * 128 : kb * 128 + 128],
                            qT_sb[hp, pair, qb * 512 + soff : qb * 512 + 512],
                            start=True,
                            stop=True,
                        )
                    expt = pool_exp.tile([128, 2, 512], BF16, tag="expt")
                    if r <= 0:
                        nc.scalar.activation(
                            out=expt.rearrange("p h q -> p (h q)"),
                            in_=sps.rearrange("p h q -> p (h q)"),
                            func=EXPF,
                            scale=SCEXP,
                        )
                    else:
                        nc.scalar.activation(
                            out=expt[:, :, soff:512],
                            in_=sps[:, :, soff:512],
                            func=EXPF,
                            scale=SCEXP,
                        )
                    if r >= 0:
                        # within-tile causal mask on the diagonal strip; the
                        # last diagonals gate the pair-end attnV drain, so
                        # run them on DVE (no Q7 launch latency)
                        tri_eng = nc.vector
                        tri_eng.tensor_tensor(
                            out=expt[:, :, soff : soff + 128],
                            in0=expt[:, :, soff : soff + 128],
                            in1=tri2_sb,
                            op=MUL,
                        )
                    lagged.append((expt, kb, r))
                    if len(lagged) > 5:
                        attnv(*lagged.popleft())
                    if kb == 1 and norm_q:
                        norm_q.popleft()()
                    if kb == 2:
                        alloc_acc()
                    if epi_q and kb == 3:
                        epi_q.popleft()()
                    if kb >= 6 and staged:
                        queue.extend(staged)
                        staged.clear()
                    pump()
                queue.extend(staged)
                staged.clear()
                drain_due(2 * qb + pair + 0.5)
                last = qb == NQB - 1 and pair == 1
                if last:
                    # h-major drain: head 0 finishes first so its normalize
                    # overlaps head 1's remaining matmuls
                    tail_kbs = list(lagged)
                    lagged.clear()
                else:
                    while lagged:
                        attnv(*lagged.popleft())
                while epi_q:
                    epi_q.popleft()()

                # normalize off the accumulators: batched reciprocal of the
                # ones-column denominators, then fused mult-evacuate to bf16.
                # Deferred into the next pair's kb==1 so the DVE chain never
                # sits at the PE queue head during the pair transition.
                attnq = pool_sm.tile([128, 4, 2, HD], BF16, tag="attnq")
                rec = pool_sm.tile([128, 2, 4], F32, tag="rec")

                def norm_h(h, accv=accv, attnq=attnq, rec=rec):
                    nc.vector.reciprocal(
                        out=rec[:, h, :],
                        in_=accv[h][:, :, HD : HD + 1].rearrange("p a c -> p (a c)"),
                    )
                    nc.vector.tensor_tensor(
                        out=attnq[:, :, h, :],
                        in0=accv[h][:, :, 0:HD],
                        in1=rec[:, h, :].broadcast_to([128, 4, HD]),
                        op=MUL,
                    )

                def norm_emit():
                    norm_h(0)
                    norm_h(1)

                ep = epilogue_tail(attnq, pair, qb)
                if last:
                    for h in range(2):
                        for expt_, kb_, r_ in tail_kbs:
                            for qt in range(max(r_, 0), 4):
                                nc.tensor.matmul(
                                    accv[h][:, qt, :],
                                    expt_[:, h, qt * 128 : qt * 128 + 128],
                                    vaug_sb[:, kb_, 2 * pair + h, :],
                                    start=False,
                                    stop=(kb_ == 4 * qb + qt),
                                    skip_group_check=True,
                                )
                        norm_h(h)
                    ep()
                else:
                    norm_q.append(norm_emit)
                    epi_q.append(ep)

            if qb == NQB - 1:
                for qs in range(4):
                    reserve.append(oproj_tail(qb * 512 + qs * 128))
            future_oproj -= 4 * 900

        attn_ctx.close()
        pp_tail = ctx.enter_context(
            tc.tile_pool(name="pp_tail", bufs=4, space="PSUM")
        )
        oproj_pool[0] = pp_tail
        while reserve:
            reserve.popleft()()
        while queue:
            _, _, emit = queue.popleft()
            emit()

    nc.compile()
    return nc


_NC = None


def _get_nc():
    global _NC
    if _NC is None:
        _NC = _build()
    return _NC


def kernel(x, mask, Wqkv, bqkv, Wout, bout):
    x = np.asarray(x, dtype=np.float32)
    Wqkv = np.asarray(Wqkv, dtype=np.float32)
    bqkv = np.asarray(bqkv, dtype=np.float32)
    Wout = np.asarray(Wout, dtype=np.float32)
    bout = np.asarray(bout, dtype=np.float32)
    assert not np.any(bqkv), "nonzero bqkv not supported by this kernel"

    import ml_dtypes

    bf16 = ml_dtypes.bfloat16
    f8 = ml_dtypes.float8_e4m3

    def hilo(a):
        hi = a.astype(f8)
        lo = (a - hi.astype(np.float32)).astype(f8)
        return np.ascontiguousarray(hi), np.ascontiguousarray(lo)

    # host-side layout prep; x and the qkv weights ship as fp8 hi/lo pairs
    xhis, xlos = [], []
    for b in range(B):
        xt = x[b].T.reshape(NDT, 128, S).transpose(1, 0, 2)  # [128, 8, 2048]
        hi, lo = hilo(xt)
        xhis.append(hi)
        xlos.append(lo)
    tri = np.triu(np.ones((128, 128), dtype=np.float32)).astype(bf16)
    tri2 = np.ascontiguousarray(np.stack([tri, tri], axis=1))  # [128, 2, 128]
    identv = np.ascontiguousarray(np.eye(128, dtype=np.float32).astype(bf16))

    def wslice(j, g):  # j: 0=q,1=k,2=v -> hi/lo [128, 8, 256] fp8
        cols = Wqkv[:, j * D + g * FL : j * D + (g + 1) * FL] * WSCALE
        wt = cols.reshape(NDT, 128, FL).transpose(1, 0, 2)
        return hilo(wt)

    in_maps = []
    for c in range(8):
        b, g = c // G, c % G
        whq_, wlq_ = wslice(0, g)
        whk_, wlk_ = wslice(1, g)
        whv_, wlv_ = wslice(2, g)
        wo = Wout[g * FL : (g + 1) * FL, :]  # [256, 1024]
        in_maps.append(
            {
                "xhi": xhis[b],
                "xlo": xlos[b],
                "whq": whq_,
                "wlq": wlq_,
                "whk": whk_,
                "wlk": wlk_,
                "whv": whv_,
                "wlv": wlv_,
                "wout": np.ascontiguousarray(
                    wo.reshape(2, 128, D).transpose(1, 0, 2).astype(bf16)
                ),
                "tri2": tri2,
                "ident": identv,
            }
        )

    nc = _get_nc()
    # axon terminals occasionally flake: transient NRT_EXEC_UNIT errors
    # (caught+retried) but also rare silent numeric corruption on a core.
    # Dispatch twice and cross-check; on mismatch, a third run breaks the
    # tie (device execution is deterministic, so good runs agree exactly).
    import time as _time

    def dispatch():
        for attempt in range(3):
            try:
                res = run_bass_kernel_spmd(nc, in_maps, core_ids=list(range(8)))
                break
            except Exception:
                if attempt == 2:
                    raise
                _time.sleep(2.0)
        out = np.empty((B, S, D), dtype=np.float32)
        for b in range(B):
            acc = res.results[b * G]["out_p"].astype(np.float32).copy()
            for g in range(1, G):
                acc += res.results[b * G + g]["out_p"]
            out[b] = acc + bout[None, :]
        return out

    def close(a, b):
        return np.linalg.norm(a - b) <= 1e-4 * np.linalg.norm(a)

    out1 = dispatch()
    out2 = dispatch()
    if close(out1, out2):
        return out1
    out3 = dispatch()
    if close(out1, out3):
        return out1
    if close(out2, out3):
        return out2
    return out3


# revision 9
# speedup vs baseline: 1.0132x; 1.0004x over previous
"""Causal self-attention on 8 trn2 NeuronCores — v2.

Sharding: core c = (b, g) with b = c // 4 (batch), g = c % 4 (head group of
4 heads).  Each core computes q/k/v projections for its 4 heads, causal
attention, and a partial out-projection (its 256 rows of Wout).  Host sums
the 4 partials per batch and adds bout.

v2 structural changes vs v1:
  * q/k/v projections run as fp8e4m3 DoubleRow matmuls with hi+lo error
    compensation (W' = 32*W split into Whi+Wlo, x into xhi+xlo; the three
    products Whi.xhi + Whi.xlo + Wlo.xhi land in one f32 psum).  25% fewer
    PE cycles than bf16 at ~bf16 accuracy; the 32x scale folds into the
    exp scale (q,k) and the v evacuation (x 1/32).
  * attnV swaps moving/stationary: expt tiles [128k x 128q] are the
    stationary operand, vaug [128k x 65] the moving one, accumulating into
    per-head psum accumulators [q, 4qt, 65] — 65-cycle matmuls instead of
    width-cycle ones (2x fewer PE cycles), with the softmax denominator in
    column 64 via the vaug ones-column.
  * normalization fuses into the accumulator evacuation (tensor_tensor with
    a stride-0-broadcast reciprocal), then PE transposes [q, f] -> [f, q]
    tiles through identity is_transpose matmuls for the out-projection.
  * psum accumulation uses one start=True per 2KB bank zero-region; sibling
    chains open start=False and rely on pending-zero (all psum tags are
    bank-sized so regions never straddle tiles).

Layouts on device:
  xhi/xlo  [128, 8, 2048] fp8   x[b]^T, d-tile major
  wh*/wl*  [128, 8, 256]  fp8   32*W columns for this group, d-tile major
  qT/kT    [128, 2, 2048] bf16  [2 heads x 64 hd][pair][pos], carries x32
  vaug     [128, 16, 4, 65] bf16  per k-tile, per head: 64 v-cols + ones
  expt     [128, 2, 512]  bf16  exp(scores^T) per k-tile, [k][head][q]
  attnT    [128, 2, 2048] bf16  normalized attn, features on partitions
"""

import sys

if "/opt/trn_rl_repo" not in sys.path:
    sys.path.insert(0, "/opt/trn_rl_repo")

import numpy as np

import concourse.mybir as mybir
import concourse.tile as tile
from concourse import bacc
from concourse.bass_utils import run_bass_kernel_spmd
from concourse.vector_clock import ScopedClock, VectorClock

B, S, D, H, HD = 2, 2048, 1024, 16, 64
G = 4            # head groups (cores per batch)
HL = H // G      # heads per core = 4
FL = HL * HD     # local features = 256
NQB = S // 512   # 4 q-blocks of 512
NST = S // 128   # 16 s-tiles of 128
NDT = D // 128   # 8 d-tiles
NDP = NDT // 2   # 4 d-tile pairs for DoubleRow

F32 = mybir.dt.float32
BF16 = mybir.dt.bfloat16
F8 = mybir.dt.float8e4
EXPF = mybir.ActivationFunctionType.Exp
DR = mybir.MatmulPerfMode.DoubleRow
MUL = mybir.AluOpType.mult

WSCALE = 32.0                    # W' = 32*W for fp8 hi/lo headroom
SCEXP = 0.125 / (WSCALE * WSCALE)  # exp scale: 1/sqrt(HD) / (32*32)


class SplitDrainTC(tile.TileContext):
    """This walrus build rejects >1 sync wait on an SP Drain; emit one
    drain per live proc instead of a single fat one."""

    def _drain_and_barrier(self, tick_clock, wait_clock):
        g = tick_clock.global_clock
        n = len(g)
        live = [(p, g[p]) for p in range(n) if g[p] > 0]
        if not live:
            self.nc.sync.drain()
        for p, t in live:
            vec = [0] * n
            vec[p] = t
            d = self.nc.sync.drain()
            wait_clock.add_sem_waits(d.ins, ScopedClock({None: VectorClock(vec)}))
        self.nc.all_engine_barrier()
        assert self.sems is not None
        popped = self.nc._tile_sem_poison_stack.pop()
        assert popped is self._sem_poison
        self.nc.clear_and_free_semaphores(list(self.sems.allocated().values()))
        self.nc.all_engine_barrier()


def _build(debug=False):
    nc = bacc.Bacc()
    xhi = nc.declare_dram_parameter("xhi", [128, NDT, S], F8, isOutput=False)
    xlo = nc.declare_dram_parameter("xlo", [128, NDT, S], F8, isOutput=False)
    whq = nc.declare_dram_parameter("whq", [128, NDT, FL], F8, isOutput=False)
    wlq = nc.declare_dram_parameter("wlq", [128, NDT, FL], F8, isOutput=False)
    whk = nc.declare_dram_parameter("whk", [128, NDT, FL], F8, isOutput=False)
    wlk = nc.declare_dram_parameter("wlk", [128, NDT, FL], F8, isOutput=False)
    whv = nc.declare_dram_parameter("whv", [128, NDT, FL], F8, isOutput=False)
    wlv = nc.declare_dram_parameter("wlv", [128, NDT, FL], F8, isOutput=False)
    wout = nc.declare_dram_parameter("wout", [128, 2, D], BF16, isOutput=False)
    tri2 = nc.declare_dram_parameter("tri2", [128, 2, 128], BF16, isOutput=False)
    ident = nc.declare_dram_parameter("ident", [128, 128], BF16, isOutput=False)
    out_p = nc.declare_dram_parameter("out_p", [S, D], BF16, isOutput=True)

    from collections import deque
    from contextlib import ExitStack

    with SplitDrainTC(nc) as tc, ExitStack() as ctx:
        consts = ctx.enter_context(tc.tile_pool(name="consts", bufs=1))
        pp_fill = ctx.enter_context(tc.tile_pool(name="pp_fill", bufs=2, space="PSUM"))
        attn_ctx = ExitStack()
        pp_sc = attn_ctx.enter_context(tc.tile_pool(name="pp_sc", bufs=2, space="PSUM"))
        pp_acc = attn_ctx.enter_context(
            tc.tile_pool(name="pp_acc", bufs=1, space="PSUM")
        )
        oproj_pool = [pp_fill]
        pool_exp = ctx.enter_context(tc.tile_pool(name="pool_exp", bufs=7))
        pool_out = ctx.enter_context(tc.tile_pool(name="pool_out", bufs=5))
        pool_sm = ctx.enter_context(tc.tile_pool(name="pool_sm", bufs=4))

        xhi_sb = consts.tile([128, NDT, S], F8)
        xlo_sb = consts.tile([128, NDT, S], F8)
        whq_sb = consts.tile([128, NDT, FL], F8)
        wlq_sb = consts.tile([128, NDT, FL], F8)
        whk_sb = consts.tile([128, NDT, FL], F8)
        wlk_sb = consts.tile([128, NDT, FL], F8)
        whv_sb = consts.tile([128, NDT, FL], F8)
        wlv_sb = consts.tile([128, NDT, FL], F8)
        wout_sb = consts.tile([128, 2, D], BF16)
        tri2_sb = consts.tile([128, 2, 128], BF16)
        id_sb = consts.tile([128, 128], BF16)
        qT_sb = consts.tile([128, 2, S], BF16)
        kT_sb = consts.tile([128, 2, S], BF16)
        vaug_sb = consts.tile([128, NST, HL, HD + 1], BF16)
        attnT_sb = consts.tile([128, 2, S], BF16)

        # PE clock-ramp warmup: dummy matmuls on zeroed SBUF while the
        # first DMAs land, so real matmuls start at full clock.
        nc.vector.memset(attnT_sb[:, 0, 0:256], 0.0)
        for i in range(64):
            wps = pp_fill.tile([128, 512], F32, tag="fill")
            nc.tensor.matmul(
                wps[:, 0:128],
                attnT_sb[:, 0, 0:128],
                attnT_sb[:, 0, 128:256],
                start=True,
                stop=True,
            )

        # DMA order matters: first matmuls need wq hi/lo and the first
        # s-block of xhi/xlo; weights issue from the (idle-at-start) ACT
        # queue so their descriptor generation runs parallel to the x
        # stream on SP.
        nc.scalar.dma_start(out=whq_sb, in_=whq[:])
        nc.scalar.dma_start(out=wlq_sb, in_=wlq[:])
        nc.sync.dma_start(out=xhi_sb[:, :, 0:512], in_=xhi[:, :, 0:512])
        nc.scalar.dma_start(out=whk_sb, in_=whk[:])
        nc.scalar.dma_start(out=wlk_sb, in_=wlk[:])
        nc.scalar.dma_start(out=tri2_sb, in_=tri2[:])
        nc.sync.dma_start(out=xlo_sb[:, :, 0:512], in_=xlo[:, :, 0:512])
        nc.scalar.dma_start(out=whv_sb, in_=whv[:])
        nc.scalar.dma_start(out=wlv_sb, in_=wlv[:])
        nc.scalar.dma_start(out=id_sb, in_=ident[:])
        nc.sync.dma_start(out=xhi_sb[:, :, 512:1024], in_=xhi[:, :, 512:1024])
        nc.sync.dma_start(out=xlo_sb[:, :, 512:1024], in_=xlo[:, :, 512:1024])
        nc.scalar.dma_start(out=wout_sb, in_=wout[:])
        nc.sync.dma_start(out=xhi_sb[:, :, 1024:S], in_=xhi[:, :, 1024:S])
        nc.sync.dma_start(out=xlo_sb[:, :, 1024:S], in_=xlo[:, :, 1024:S])
        # ACT spline-table preload for Exp, after the weight DMA issues so
        # it doesn't delay them on the ACT queue
        warm = pool_sm.tile([1, 1], F32, tag="warm")
        nc.vector.memset(warm, 0.0)
        nc.scalar.activation(out=warm, in_=warm, func=EXPF)
        # ones columns of vaug (constant across the run)
        nc.gpsimd.memset(vaug_sb[:, :, :, HD : HD + 1], 1.0)

        # ---- chunk emitters (projections / out-proj used as PE filler) ----
        def qkT_chunk(wh_sb, wl_sb, dst, ft, sb_):
            # 512 positions of one 128-feature column tile of q or k:
            # 2 pos-chunks x (4 d-pairs x 3 comp terms) DoubleRow matmuls.
            def emit():
                ps = pp_fill.tile([128, 512], F32, tag="fill")
                for c in range(2):
                    p0 = sb_ * 512 + c * 256
                    first = True
                    for dp in range(NDP):
                        dsl = slice(2 * dp, 2 * dp + 2)
                        fsl = slice(ft * 128, ft * 128 + 128)
                        for wmat, xmat in (
                            (wh_sb, xhi_sb),
                            (wl_sb, xhi_sb),
                            (wh_sb, xlo_sb),
                        ):
                            nc.tensor.matmul(
                                ps[:, c * 256 : c * 256 + 256],
                                wmat[:, dsl, fsl],
                                xmat[:, dsl, p0 : p0 + 256],
                                start=first,
                                stop=(dp == NDP - 1 and xmat is xlo_sb),
                                perf_mode=DR,
                                skip_group_check=True,
                            )
                            first = False
                nc.vector.tensor_copy(
                    out=dst[:, ft, sb_ * 512 : sb_ * 512 + 512], in_=ps[:, 0:512]
                )

            return emit

        def v_chunk(st):
            def emit():
                ps = pp_fill.tile([128, 512], F32, tag="fill")
                first = True
                for dp in range(NDP):
                    dsl = slice(2 * dp, 2 * dp + 2)
                    psl = slice(st * 128, st * 128 + 128)
                    for xmat, wmat in (
                        (xhi_sb, whv_sb),
                        (xlo_sb, whv_sb),
                        (xhi_sb, wlv_sb),
                    ):
                        nc.tensor.matmul(
                            ps[:, 0:FL],
                            xmat[:, dsl, psl],
                            wmat[:, dsl, :],
                            start=first,
                            stop=(dp == NDP - 1 and wmat is wlv_sb),
                            perf_mode=DR,
                            skip_group_check=True,
                        )
                        first = False
                # evacuate with the 1/32 descale (W' = 32*W)
                nc.vector.tensor_scalar(
                    out=vaug_sb[:, st, :, 0:HD],
                    in0=ps[:, 0:FL].rearrange("p (h e) -> p h e", h=HL),
                    scalar1=1.0 / WSCALE,
                    scalar2=None,
                    op0=MUL,
                )

            return emit

        def oproj_tail(q0):
            # tail variant: both 512-col halves of a q-tile, one combined
            # 2KB DMA; DVE and ACT each evacuate one half
            def emit():
                out_t = pool_out.tile([128, 2, 512], BF16, tag="outw")
                for dc in range(2):
                    ops = oproj_pool[0].tile([128, 512], F32, tag="fill")
                    for ft in range(2):
                        nc.tensor.matmul(
                            ops[:, 0:512],
                            attnT_sb[:, ft, q0 : q0 + 128],
                            wout_sb[:, ft, dc * 512 : dc * 512 + 512],
                            start=(ft == 0),
                            stop=(ft == 1),
                        )
                    if dc == 0:
                        nc.vector.tensor_copy(out=out_t[:, 0, :], in_=ops[:, 0:512])
                    else:
                        nc.scalar.copy(out=out_t[:, 1, :], in_=ops[:, 0:512])
                nc.sync.dma_start(
                    out=out_p[q0 : q0 + 128, :],
                    in_=out_t.rearrange("p a b -> p (a b)"),
                )

            return emit

        def oproj_half(q0, dc, late=False):
            def emit():
                ops = oproj_pool[0].tile([128, 512], F32, tag="fill")
                for ft in range(2):
                    nc.tensor.matmul(
                        ops[:, 0:512],
                        attnT_sb[:, ft, q0 : q0 + 128],
                        wout_sb[:, ft, dc * 512 : dc * 512 + 512],
                        start=(ft == 0),
                        stop=(ft == 1),
                    )
                out_t = pool_out.tile([128, 512], BF16, tag="out")
                if late and dc == 1:
                    # post-attention: ACT is idle, split the evacuations
                    nc.scalar.copy(out=out_t, in_=ops[:, 0:512])
                else:
                    nc.vector.tensor_copy(out=out_t, in_=ops[:, 0:512])
                nc.sync.dma_start(
                    out=out_p[q0 : q0 + 128, dc * 512 : dc * 512 + 512], in_=out_t
                )

            return emit

        # filler queue: (deadline, cost_ns, emit_fn); FIFO order respects deps.
        # deadline units: 2*qb + pair (+0.5 for "before this pair's attnV
        # drain"); drain_due forces everything due at each boundary.
        queue = deque()
        reserve = deque()
        for qb in range(NQB):
            for wh_sb, wl_sb, dst in (
                (whq_sb, wlq_sb, qT_sb),
                (whk_sb, wlk_sb, kT_sb),
            ):
                if qb > 0:
                    queue.append(
                        (2 * qb - 1.25, 1300, qkT_chunk(wh_sb, wl_sb, dst, 0, qb))
                    )
            for st in range(4 * qb, 4 * qb + 4):
                queue.append((2 * qb + 0.5, 650, v_chunk(st)))
            for wh_sb, wl_sb, dst in (
                (whq_sb, wlq_sb, qT_sb),
                (whk_sb, wlk_sb, kT_sb),
            ):
                queue.append(
                    (
                        max(0.75, 2 * qb - 0.25),
                        1300,
                        qkT_chunk(wh_sb, wl_sb, dst, 1, qb),
                    )
                )

        # Adaptive pump: spread remaining filler cost over remaining attention
        # steps so late q-blocks (which have no projections left) still get
        # out-proj chunks as PE filler.
        total_steps = sum(2 * (4 * qb + 4) for qb in range(NQB))  # 80
        future_oproj = 4 * NQB * 900
        step_no = 0

        tokens = 0.0
        PUMP_RATE = 355.0  # ~per-step PE deficit vs the ACT exp stream

        def pump():
            nonlocal step_no, future_oproj, tokens
            step_no += 1
            tokens += PUMP_RATE
            while queue and tokens >= queue[0][1]:
                _, cost, emit = queue.popleft()
                emit()
                tokens -= cost

        def drain_due(qb):
            while queue and queue[0][0] <= qb:
                _, _, emit = queue.popleft()
                emit()

        # ---- prologue: only what (qb0, pair0) scores need; the rest
        # streams in as filler during pair0 ----
        qkT_chunk(whq_sb, wlq_sb, qT_sb, 0, 0)()
        qkT_chunk(whk_sb, wlk_sb, kT_sb, 0, 0)()

        # deferred per-(qb,pair) epilogue (transposes + attnT evac), emitted
        # a few kb-steps into the NEXT pair so PE never waits on the DVE
        # normalize chain
        epi_q = deque()
        norm_q = deque()
        staged = deque()

        def epilogue_tail(attnq, pair_, qb_):
            def emit():
                tp = pp_fill.tile([128, 512], F32, tag="fill")
                tpb = tp.bitcast(BF16)
                for qt in range(4):
                    nc.tensor.matmul(
                        tpb[:, qt * 128 : qt * 128 + 128],
                        attnq[:, qt, :, :].rearrange("p h f -> p (h f)"),
                        id_sb,
                        start=(qt == 0),
                        stop=(qt == 3),
                        is_transpose=True,
                        skip_group_check=True,
                    )
                nc.vector.tensor_copy(
                    out=attnT_sb[:, pair_, qb_ * 512 : qb_ * 512 + 512],
                    in_=tpb[:, 0:512],
                )
                if pair_ == 1:
                    # attnT for qb_ is complete -> its out-proj becomes
                    # filler, but hold it a few kb-steps so the pump can't
                    # pop it while the attnT evacuation is still in flight.
                    for qs_ in range(4):
                        for dc_ in range(2):
                            staged.append(
                                (
                                    100,
                                    450,
                                    oproj_half(
                                        qb_ * 512 + qs_ * 128,
                                        dc_,
                                        late=(qb_ == NQB - 1),
                                    ),
                                )
                            )

            return emit

        # ---- attention (scores -> exp/mask -> lagged swapped attnV) ----
        for qb in range(NQB):
            for pair in range(2):
                drain_due(2 * qb + pair)
                nkb = 4 * qb + 4
                # acc tiles are allocated lazily at kb==2, after the previous
                # pair's deferred normalize has been emitted (pool WAR
                # tracking needs readers emitted before the next allocation)
                accv = [None, None]

                def alloc_acc(accv=accv):
                    for h in range(2):
                        a = pp_acc.tile([128, 512], F32, tag=f"acc{h}")
                        accv[h] = a[:, 0:260].rearrange("p (a c) -> p a c", c=HD + 1)

                lagged = deque()  # expt tiles awaiting their attnV matmuls

                def attnv(expt, kb, r, accv=accv, pair=pair, qb=qb):
                    for qt in range(max(r, 0), 4):
                        for h in range(2):
                            nc.tensor.matmul(
                                accv[h][:, qt, :],
                                expt[:, h, qt * 128 : qt * 128 + 128],
                                vaug_sb[:, kb, 2 * pair + h, :],
                                start=(kb == 0 and qt == max(r, 0)),
                                stop=(kb == 4 * qb + qt),
                                skip_group_check=True,
                            )

                for kb in range(nkb):
                    r = kb - 4 * qb
                    soff = 128 * max(r, 0)
                    sps = pp_sc.tile([128, 2, 512], F32, tag="ps")
                    for h in range(2):
                        hp = slice(64 * h, 64 * h + 64)
                        nc.tensor.matmul(
                            sps[:, h, soff:512],
                            kT_sb[hp, pair, kb * 128 : kb * 128 + 128],
                            qT_sb[hp, pair, qb * 512 + soff : qb * 512 + 512],
                            start=True,
                            stop=True,
                        )
                    expt = pool_exp.tile([128, 2, 512], BF16, tag="expt")
                    if r <= 0:
                        nc.scalar.activation(
                            out=expt.rearrange("p h q -> p (h q)"),
                            in_=sps.rearrange("p h q -> p (h q)"),
                            func=EXPF,
                            scale=SCEXP,
                        )
                    else:
                        nc.scalar.activation(
                            out=expt[:, :, soff:512],
                            in_=sps[:, :, soff:512],
                            func=EXPF,
                            scale=SCEXP,
                        )
                    if r >= 0:
                        # within-tile causal mask on the diagonal strip; the
                        # last diagonals gate the pair-end attnV drain, so
                        # run them on DVE (no Q7 launch latency)
                        tri_eng = nc.vector
                        tri_eng.tensor_tensor(
                            out=expt[:, :, soff : soff + 128],
                            in0=expt[:, :, soff : soff + 128],
                            in1=tri2_sb,
                            op=MUL,
                        )
                    lagged.append((expt, kb, r))
                    if len(lagged) > 6:
                        attnv(*lagged.popleft())
                    if kb == 1 and norm_q:
                        norm_q.popleft()()
                    if kb == 2:
                        alloc_acc()
                    if epi_q and kb == 7:
                        epi_q.popleft()()
                    if kb >= 9 and staged:
                        queue.extend(staged)
                        staged.clear()
                    pump()
                queue.extend(staged)
                staged.clear()
                drain_due(2 * qb + pair + 0.5)
                last = qb == NQB - 1 and pair == 1
                if last:
                    # h-major drain: head 0 finishes first so its normalize
                    # overlaps head 1's remaining matmuls
                    tail_kbs = list(lagged)
                    lagged.clear()
                else:
                    while lagged:
                        attnv(*lagged.popleft())
                while epi_q:
                    epi_q.popleft()()

                # normalize off the accumulators: batched reciprocal of the
                # ones-column denominators, then fused mult-evacuate to bf16.
                # Deferred into the next pair's kb==1 so the DVE chain never
                # sits at the PE queue head during the pair transition.
                attnq = pool_sm.tile([128, 4, 2, HD], BF16, tag="attnq")
                rec = pool_sm.tile([128, 2, 4], F32, tag="rec")

                def norm_h(h, accv=accv, attnq=attnq, rec=rec):
                    nc.vector.reciprocal(
                        out=rec[:, h, :],
                        in_=accv[h][:, :, HD : HD + 1].rearrange("p a c -> p (a c)"),
                    )
                    nc.vector.tensor_tensor(
                        out=attnq[:, :, h, :],
                        in0=accv[h][:, :, 0:HD],
                        in1=rec[:, h, :].broadcast_to([128, 4, HD]),
                        op=MUL,
                    )

                def norm_emit():
                    norm_h(0)
                    norm_h(1)

                ep = epilogue_tail(attnq, pair, qb)
                if last:
                    for h in range(2):
                        for expt_, kb_, r_ in tail_kbs:
                            for qt in range(max(r_, 0), 4):
                                nc.tensor.matmul(
                                    accv[h][:, qt, :],
                                    expt_[:, h, qt * 128 : qt * 128 + 128],
                                    vaug_sb[:, kb_, 2 * pair + h, :],
                                    start=False,
                                    stop=(kb_ == 4 * qb + qt),
                                    skip_group_check=True,
                                )
                        norm_h(h)
                    ep()
                else:
                    norm_q.append(norm_emit)
                    epi_q.append(ep)

            if qb == NQB - 1:
                for qs in range(4):
                    reserve.append(oproj_tail(qb * 512 + qs * 128))
            future_oproj -= 4 * 900

        attn_ctx.close()
        pp_tail = ctx.enter_context(
            tc.tile_pool(name="pp_tail", bufs=4, space="PSUM")
        )
        oproj_pool[0] = pp_tail
        while reserve:
            reserve.popleft()()
        while queue:
            _, _, emit = queue.popleft()
            emit()

    nc.compile()
    return nc


_NC = None


def _get_nc():
    global _NC
    if _NC is None:
        _NC = _build()
    return _NC


def kernel(x, mask, Wqkv, bqkv, Wout, bout):
    x = np.asarray(x, dtype=np.float32)
    Wqkv = np.asarray(Wqkv, dtype=np.float32)
    bqkv = np.asarray(bqkv, dtype=np.float32)
    Wout = np.asarray(Wout, dtype=np.float32)
    bout = np.asarray(bout, dtype=np.float32)
    assert not np.any(bqkv), "nonzero bqkv not supported by this kernel"

    import ml_dtypes

    bf16 = ml_dtypes.bfloat16
    f8 = ml_dtypes.float8_e4m3

    def hilo(a):
        hi = a.astype(f8)
        lo = (a - hi.astype(np.float32)).astype(f8)
        return np.ascontiguousarray(hi), np.ascontiguousarray(lo)

    # host-side layout prep; x and the qkv weights ship as fp8 hi/lo pairs
    xhis, xlos = [], []
    for b in range(B):
        xt = x[b].T.reshape(NDT, 128, S).transpose(1, 0, 2)  # [128, 8, 2048]
        hi, lo = hilo(xt)
        xhis.append(hi)
        xlos.append(lo)
    tri = np.triu(np.ones((128, 128), dtype=np.float32)).astype(bf16)
    tri2 = np.ascontiguousarray(np.stack([tri, tri], axis=1))  # [128, 2, 128]
    identv = np.ascontiguousarray(np.eye(128, dtype=np.float32).astype(bf16))

    def wslice(j, g):  # j: 0=q,1=k,2=v -> hi/lo [128, 8, 256] fp8
        cols = Wqkv[:, j * D + g * FL : j * D + (g + 1) * FL] * WSCALE
        wt = cols.reshape(NDT, 128, FL).transpose(1, 0, 2)
        return hilo(wt)

    in_maps = []
    for c in range(8):
        b, g = c // G, c % G
        whq_, wlq_ = wslice(0, g)
        whk_, wlk_ = wslice(1, g)
        whv_, wlv_ = wslice(2, g)
        wo = Wout[g * FL : (g + 1) * FL, :]  # [256, 1024]
        in_maps.append(
            {
                "xhi": xhis[b],
                "xlo": xlos[b],
                "whq": whq_,
                "wlq": wlq_,
                "whk": whk_,
                "wlk": wlk_,
                "whv": whv_,
                "wlv": wlv_,
                "wout": np.ascontiguousarray(
                    wo.reshape(2, 128, D).transpose(1, 0, 2).astype(bf16)
                ),
                "tri2": tri2,
                "ident": identv,
            }
        )

    nc = _get_nc()
    # axon terminals occasionally flake: transient NRT_EXEC_UNIT errors
    # (caught+retried) but also rare silent numeric corruption on a core.
    # Dispatch twice and cross-check; on mismatch, a third run breaks the
    # tie (device execution is deterministic, so good runs agree exactly).
    import time as _time

    def dispatch():
        for attempt in range(3):
            try:
                res = run_bass_kernel_spmd(nc, in_maps, core_ids=list(range(8)))
                break
            except Exception:
                if attempt == 2:
                    raise
                _time.sleep(2.0)
        out = np.empty((B, S, D), dtype=np.float32)
        for b in range(B):
            acc = res.results[b * G]["out_p"].astype(np.float32).copy()
            for g in range(1, G):
                acc += res.results[b * G + g]["out_p"]
            out[b] = acc + bout[None, :]
        return out

    def close(a, b):
        return np.linalg.norm(a - b) <= 1e-4 * np.linalg.norm(a)

    out1 = dispatch()
    out2 = dispatch()
    if close(out1, out2):
        return out1
    out3 = dispatch()
    if close(out1, out3):
        return out1
    if close(out2, out3):
        return out2
    return out3


# revision 10
# speedup vs baseline: 1.0151x; 1.0019x over previous
"""Causal self-attention on 8 trn2 NeuronCores — v2.

Sharding: core c = (b, g) with b = c // 4 (batch), g = c % 4 (head group of
4 heads).  Each core computes q/k/v projections for its 4 heads, causal
attention, and a partial out-projection (its 256 rows of Wout).  Host sums
the 4 partials per batch and adds bout.

v2 structural changes vs v1:
  * q/k/v projections run as fp8e4m3 DoubleRow matmuls with hi+lo error
    compensation (W' = 32*W split into Whi+Wlo, x into xhi+xlo; the three
    products Whi.xhi + Whi.xlo + Wlo.xhi land in one f32 psum).  25% fewer
    PE cycles than bf16 at ~bf16 accuracy; the 32x scale folds into the
    exp scale (q,k) and the v evacuation (x 1/32).
  * attnV swaps moving/stationary: expt tiles [128k x 128q] are the
    stationary operand, vaug [128k x 65] the moving one, accumulating into
    per-head psum accumulators [q, 4qt, 65] — 65-cycle matmuls instead of
    width-cycle ones (2x fewer PE cycles), with the softmax denominator in
    column 64 via the vaug ones-column.
  * normalization fuses into the accumulator evacuation (tensor_tensor with
    a stride-0-broadcast reciprocal), then PE transposes [q, f] -> [f, q]
    tiles through identity is_transpose matmuls for the out-projection.
  * psum accumulation uses one start=True per 2KB bank zero-region; sibling
    chains open start=False and rely on pending-zero (all psum tags are
    bank-sized so regions never straddle tiles).

Layouts on device:
  xhi/xlo  [128, 8, 2048] fp8   x[b]^T, d-tile major
  wh*/wl*  [128, 8, 256]  fp8   32*W columns for this group, d-tile major
  qT/kT    [128, 2, 2048] bf16  [2 heads x 64 hd][pair][pos], carries x32
  vaug     [128, 16, 4, 65] bf16  per k-tile, per head: 64 v-cols + ones
  expt     [128, 2, 512]  bf16  exp(scores^T) per k-tile, [k][head][q]
  attnT    [128, 2, 2048] bf16  normalized attn, features on partitions
"""

import sys

if "/opt/trn_rl_repo" not in sys.path:
    sys.path.insert(0, "/opt/trn_rl_repo")

import numpy as np

import concourse.mybir as mybir
import concourse.tile as tile
from concourse import bacc
from concourse.bass_utils import run_bass_kernel_spmd
from concourse.vector_clock import ScopedClock, VectorClock

B, S, D, H, HD = 2, 2048, 1024, 16, 64
G = 4            # head groups (cores per batch)
HL = H // G      # heads per core = 4
FL = HL * HD     # local features = 256
NQB = S // 512   # 4 q-blocks of 512
NST = S // 128   # 16 s-tiles of 128
NDT = D // 128   # 8 d-tiles
NDP = NDT // 2   # 4 d-tile pairs for DoubleRow

F32 = mybir.dt.float32
BF16 = mybir.dt.bfloat16
F8 = mybir.dt.float8e4
EXPF = mybir.ActivationFunctionType.Exp
DR = mybir.MatmulPerfMode.DoubleRow
MUL = mybir.AluOpType.mult

WSCALE = 32.0                    # W' = 32*W for fp8 hi/lo headroom
SCEXP = 0.125 / (WSCALE * WSCALE)  # exp scale: 1/sqrt(HD) / (32*32)


class SplitDrainTC(tile.TileContext):
    """This walrus build rejects >1 sync wait on an SP Drain; emit one
    drain per live proc instead of a single fat one."""

    def _drain_and_barrier(self, tick_clock, wait_clock):
        g = tick_clock.global_clock
        n = len(g)
        live = [(p, g[p]) for p in range(n) if g[p] > 0]
        if not live:
            self.nc.sync.drain()
        for p, t in live:
            vec = [0] * n
            vec[p] = t
            d = self.nc.sync.drain()
            wait_clock.add_sem_waits(d.ins, ScopedClock({None: VectorClock(vec)}))
        self.nc.all_engine_barrier()
        assert self.sems is not None
        popped = self.nc._tile_sem_poison_stack.pop()
        assert popped is self._sem_poison
        self.nc.clear_and_free_semaphores(list(self.sems.allocated().values()))
        self.nc.all_engine_barrier()


def _build(debug=False):
    nc = bacc.Bacc()
    xhi = nc.declare_dram_parameter("xhi", [128, NDT, S], F8, isOutput=False)
    xlo = nc.declare_dram_parameter("xlo", [128, NDT, S], F8, isOutput=False)
    whq = nc.declare_dram_parameter("whq", [128, NDT, FL], F8, isOutput=False)
    wlq = nc.declare_dram_parameter("wlq", [128, NDT, FL], F8, isOutput=False)
    whk = nc.declare_dram_parameter("whk", [128, NDT, FL], F8, isOutput=False)
    wlk = nc.declare_dram_parameter("wlk", [128, NDT, FL], F8, isOutput=False)
    whv = nc.declare_dram_parameter("whv", [128, NDT, FL], F8, isOutput=False)
    wlv = nc.declare_dram_parameter("wlv", [128, NDT, FL], F8, isOutput=False)
    wout = nc.declare_dram_parameter("wout", [128, 2, D], BF16, isOutput=False)
    tri2 = nc.declare_dram_parameter("tri2", [128, 2, 128], BF16, isOutput=False)
    ident = nc.declare_dram_parameter("ident", [128, 128], BF16, isOutput=False)
    out_p = nc.declare_dram_parameter("out_p", [S, D], BF16, isOutput=True)

    from collections import deque
    from contextlib import ExitStack

    with SplitDrainTC(nc) as tc, ExitStack() as ctx:
        consts = ctx.enter_context(tc.tile_pool(name="consts", bufs=1))
        pp_fill = ctx.enter_context(tc.tile_pool(name="pp_fill", bufs=2, space="PSUM"))
        attn_ctx = ExitStack()
        pp_sc = attn_ctx.enter_context(tc.tile_pool(name="pp_sc", bufs=2, space="PSUM"))
        pp_acc = attn_ctx.enter_context(
            tc.tile_pool(name="pp_acc", bufs=1, space="PSUM")
        )
        oproj_pool = [pp_fill]
        pool_exp = ctx.enter_context(tc.tile_pool(name="pool_exp", bufs=7))
        pool_out = ctx.enter_context(tc.tile_pool(name="pool_out", bufs=5))
        pool_sm = ctx.enter_context(tc.tile_pool(name="pool_sm", bufs=4))

        xhi_sb = consts.tile([128, NDT, S], F8)
        xlo_sb = consts.tile([128, NDT, S], F8)
        whq_sb = consts.tile([128, NDT, FL], F8)
        wlq_sb = consts.tile([128, NDT, FL], F8)
        whk_sb = consts.tile([128, NDT, FL], F8)
        wlk_sb = consts.tile([128, NDT, FL], F8)
        whv_sb = consts.tile([128, NDT, FL], F8)
        wlv_sb = consts.tile([128, NDT, FL], F8)
        wout_sb = consts.tile([128, 2, D], BF16)
        tri2_sb = consts.tile([128, 2, 128], BF16)
        id_sb = consts.tile([128, 128], BF16)
        qT_sb = consts.tile([128, 2, S], BF16)
        kT_sb = consts.tile([128, 2, S], BF16)
        vaug_sb = consts.tile([128, NST, HL, HD + 1], BF16)
        attnT_sb = consts.tile([128, 2, S], BF16)

        # PE clock-ramp warmup: dummy matmuls on zeroed SBUF while the
        # first DMAs land, so real matmuls start at full clock.
        nc.vector.memset(attnT_sb[:, 0, 0:256], 0.0)
        for i in range(64):
            wps = pp_fill.tile([128, 512], F32, tag="fill")
            nc.tensor.matmul(
                wps[:, 0:128],
                attnT_sb[:, 0, 0:128],
                attnT_sb[:, 0, 128:256],
                start=True,
                stop=True,
            )

        # DMA order matters: first matmuls need wq hi/lo and the first
        # s-block of xhi/xlo; weights issue from the (idle-at-start) ACT
        # queue so their descriptor generation runs parallel to the x
        # stream on SP.
        nc.scalar.dma_start(out=whq_sb, in_=whq[:])
        nc.scalar.dma_start(out=wlq_sb, in_=wlq[:])
        nc.sync.dma_start(out=xhi_sb[:, :, 0:512], in_=xhi[:, :, 0:512])
        nc.scalar.dma_start(out=whk_sb, in_=whk[:])
        nc.scalar.dma_start(out=wlk_sb, in_=wlk[:])
        nc.scalar.dma_start(out=tri2_sb, in_=tri2[:])
        nc.sync.dma_start(out=xlo_sb[:, :, 0:512], in_=xlo[:, :, 0:512])
        nc.scalar.dma_start(out=whv_sb, in_=whv[:])
        nc.scalar.dma_start(out=wlv_sb, in_=wlv[:])
        nc.scalar.dma_start(out=id_sb, in_=ident[:])
        nc.sync.dma_start(out=xhi_sb[:, :, 512:1024], in_=xhi[:, :, 512:1024])
        nc.sync.dma_start(out=xlo_sb[:, :, 512:1024], in_=xlo[:, :, 512:1024])
        nc.scalar.dma_start(out=wout_sb, in_=wout[:])
        nc.sync.dma_start(out=xhi_sb[:, :, 1024:S], in_=xhi[:, :, 1024:S])
        nc.sync.dma_start(out=xlo_sb[:, :, 1024:S], in_=xlo[:, :, 1024:S])
        # ACT spline-table preload for Exp, after the weight DMA issues so
        # it doesn't delay them on the ACT queue
        warm = pool_sm.tile([1, 1], F32, tag="warm")
        nc.vector.memset(warm, 0.0)
        nc.scalar.activation(out=warm, in_=warm, func=EXPF)
        # ones columns of vaug (constant across the run)
        nc.gpsimd.memset(vaug_sb[:, :, :, HD : HD + 1], 1.0)

        # ---- chunk emitters (projections / out-proj used as PE filler) ----
        def qkT_chunk(wh_sb, wl_sb, dst, ft, sb_):
            # 512 positions of one 128-feature column tile of q or k:
            # 2 pos-chunks x (4 d-pairs x 3 comp terms) DoubleRow matmuls.
            def emit():
                ps = pp_fill.tile([128, 512], F32, tag="fill")
                for c in range(2):
                    p0 = sb_ * 512 + c * 256
                    first = True
                    for dp in range(NDP):
                        dsl = slice(2 * dp, 2 * dp + 2)
                        fsl = slice(ft * 128, ft * 128 + 128)
                        for wmat, xmat in (
                            (wh_sb, xhi_sb),
                            (wl_sb, xhi_sb),
                            (wh_sb, xlo_sb),
                        ):
                            nc.tensor.matmul(
                                ps[:, c * 256 : c * 256 + 256],
                                wmat[:, dsl, fsl],
                                xmat[:, dsl, p0 : p0 + 256],
                                start=first,
                                stop=(dp == NDP - 1 and xmat is xlo_sb),
                                perf_mode=DR,
                                skip_group_check=True,
                            )
                            first = False
                nc.vector.tensor_copy(
                    out=dst[:, ft, sb_ * 512 : sb_ * 512 + 512], in_=ps[:, 0:512]
                )

            return emit

        def v_chunk(st):
            def emit():
                ps = pp_fill.tile([128, 512], F32, tag="fill")
                first = True
                for dp in range(NDP):
                    dsl = slice(2 * dp, 2 * dp + 2)
                    psl = slice(st * 128, st * 128 + 128)
                    for xmat, wmat in (
                        (xhi_sb, whv_sb),
                        (xlo_sb, whv_sb),
                        (xhi_sb, wlv_sb),
                    ):
                        nc.tensor.matmul(
                            ps[:, 0:FL],
                            xmat[:, dsl, psl],
                            wmat[:, dsl, :],
                            start=first,
                            stop=(dp == NDP - 1 and wmat is wlv_sb),
                            perf_mode=DR,
                            skip_group_check=True,
                        )
                        first = False
                # evacuate with the 1/32 descale (W' = 32*W)
                nc.vector.tensor_scalar(
                    out=vaug_sb[:, st, :, 0:HD],
                    in0=ps[:, 0:FL].rearrange("p (h e) -> p h e", h=HL),
                    scalar1=1.0 / WSCALE,
                    scalar2=None,
                    op0=MUL,
                )

            return emit

        def oproj_tail(q0):
            # tail variant: both 512-col halves of a q-tile, one combined
            # 2KB DMA; DVE and ACT each evacuate one half
            def emit():
                out_t = pool_out.tile([128, 2, 512], BF16, tag="outw")
                for dc in range(2):
                    ops = oproj_pool[0].tile([128, 512], F32, tag="fill")
                    for ft in range(2):
                        nc.tensor.matmul(
                            ops[:, 0:512],
                            attnT_sb[:, ft, q0 : q0 + 128],
                            wout_sb[:, ft, dc * 512 : dc * 512 + 512],
                            start=(ft == 0),
                            stop=(ft == 1),
                        )
                    if dc == 0:
                        nc.vector.tensor_copy(out=out_t[:, 0, :], in_=ops[:, 0:512])
                    else:
                        nc.scalar.copy(out=out_t[:, 1, :], in_=ops[:, 0:512])
                nc.sync.dma_start(
                    out=out_p[q0 : q0 + 128, :],
                    in_=out_t.rearrange("p a b -> p (a b)"),
                )

            return emit

        def oproj_half(q0, dc, late=False):
            def emit():
                ops = oproj_pool[0].tile([128, 512], F32, tag="fill")
                for ft in range(2):
                    nc.tensor.matmul(
                        ops[:, 0:512],
                        attnT_sb[:, ft, q0 : q0 + 128],
                        wout_sb[:, ft, dc * 512 : dc * 512 + 512],
                        start=(ft == 0),
                        stop=(ft == 1),
                    )
                out_t = pool_out.tile([128, 512], BF16, tag="out")
                if late and dc == 1:
                    # post-attention: ACT is idle, split the evacuations
                    nc.scalar.copy(out=out_t, in_=ops[:, 0:512])
                else:
                    nc.vector.tensor_copy(out=out_t, in_=ops[:, 0:512])
                nc.sync.dma_start(
                    out=out_p[q0 : q0 + 128, dc * 512 : dc * 512 + 512], in_=out_t
                )

            return emit

        # filler queue: (deadline, cost_ns, emit_fn); FIFO order respects deps.
        # deadline units: 2*qb + pair (+0.5 for "before this pair's attnV
        # drain"); drain_due forces everything due at each boundary.
        queue = deque()
        reserve = deque()
        for qb in range(NQB):
            for wh_sb, wl_sb, dst in (
                (whq_sb, wlq_sb, qT_sb),
                (whk_sb, wlk_sb, kT_sb),
            ):
                if qb > 0:
                    queue.append(
                        (2 * qb - 1.25, 1300, qkT_chunk(wh_sb, wl_sb, dst, 0, qb))
                    )
            for st in range(4 * qb, 4 * qb + 4):
                queue.append((2 * qb + 0.5, 650, v_chunk(st)))
            for wh_sb, wl_sb, dst in (
                (whq_sb, wlq_sb, qT_sb),
                (whk_sb, wlk_sb, kT_sb),
            ):
                queue.append(
                    (
                        max(0.75, 2 * qb - 0.25),
                        1300,
                        qkT_chunk(wh_sb, wl_sb, dst, 1, qb),
                    )
                )

        # Adaptive pump: spread remaining filler cost over remaining attention
        # steps so late q-blocks (which have no projections left) still get
        # out-proj chunks as PE filler.
        total_steps = sum(2 * (4 * qb + 4) for qb in range(NQB))  # 80
        future_oproj = 4 * NQB * 900
        step_no = 0

        tokens = 0.0
        PUMP_RATE = 355.0  # ~per-step PE deficit vs the ACT exp stream

        def pump():
            nonlocal step_no, future_oproj, tokens
            step_no += 1
            tokens += PUMP_RATE
            while queue and tokens >= queue[0][1]:
                _, cost, emit = queue.popleft()
                emit()
                tokens -= cost

        def drain_due(qb):
            while queue and queue[0][0] <= qb:
                _, _, emit = queue.popleft()
                emit()

        # ---- prologue: only what (qb0, pair0) scores need; the rest
        # streams in as filler during pair0 ----
        qkT_chunk(whq_sb, wlq_sb, qT_sb, 0, 0)()
        qkT_chunk(whk_sb, wlk_sb, kT_sb, 0, 0)()

        # deferred per-(qb,pair) epilogue (transposes + attnT evac), emitted
        # a few kb-steps into the NEXT pair so PE never waits on the DVE
        # normalize chain
        epi_q = deque()
        norm_q = deque()
        staged = deque()

        def epilogue_tail(attnq, pair_, qb_):
            def emit():
                tp = pp_fill.tile([128, 512], F32, tag="fill")
                tpb = tp.bitcast(BF16)
                for qt in range(4):
                    nc.tensor.matmul(
                        tpb[:, qt * 128 : qt * 128 + 128],
                        attnq[:, qt, :, :].rearrange("p h f -> p (h f)"),
                        id_sb,
                        start=(qt == 0),
                        stop=(qt == 3),
                        is_transpose=True,
                        skip_group_check=True,
                    )
                nc.vector.tensor_copy(
                    out=attnT_sb[:, pair_, qb_ * 512 : qb_ * 512 + 512],
                    in_=tpb[:, 0:512],
                )
                if pair_ == 1:
                    # attnT for qb_ is complete -> its out-proj becomes
                    # filler, but hold it a few kb-steps so the pump can't
                    # pop it while the attnT evacuation is still in flight.
                    for qs_ in range(4):
                        for dc_ in range(2):
                            staged.append(
                                (
                                    100,
                                    450,
                                    oproj_half(
                                        qb_ * 512 + qs_ * 128,
                                        dc_,
                                        late=(qb_ == NQB - 1),
                                    ),
                                )
                            )

            return emit

        # ---- attention (scores -> exp/mask -> lagged swapped attnV) ----
        for qb in range(NQB):
            for pair in range(2):
                drain_due(2 * qb + pair)
                nkb = 4 * qb + 4
                # acc tiles are allocated lazily at kb==2, after the previous
                # pair's deferred normalize has been emitted (pool WAR
                # tracking needs readers emitted before the next allocation)
                accv = [None, None]

                def alloc_acc(accv=accv):
                    for h in range(2):
                        a = pp_acc.tile([128, 512], F32, tag=f"acc{h}")
                        accv[h] = a[:, 0:260].rearrange("p (a c) -> p a c", c=HD + 1)

                lagged = deque()  # expt tiles awaiting their attnV matmuls

                def attnv(expt, kb, r, accv=accv, pair=pair, qb=qb):
                    for qt in range(max(r, 0), 4):
                        for h in range(2):
                            nc.tensor.matmul(
                                accv[h][:, qt, :],
                                expt[:, h, qt * 128 : qt * 128 + 128],
                                vaug_sb[:, kb, 2 * pair + h, :],
                                start=(kb == 0 and qt == max(r, 0)),
                                stop=(kb == 4 * qb + qt),
                                skip_group_check=True,
                            )

                for kb in range(nkb):
                    r = kb - 4 * qb
                    soff = 128 * max(r, 0)
                    sps = pp_sc.tile([128, 2, 512], F32, tag="ps")
                    for h in range(2):
                        hp = slice(64 * h, 64 * h + 64)
                        nc.tensor.matmul(
                            sps[:, h, soff:512],
                            kT_sb[hp, pair, kb * 128 : kb * 128 + 128],
                            qT_sb[hp, pair, qb * 512 + soff : qb * 512 + 512],
                            start=True,
                            stop=True,
                        )
                    expt = pool_exp.tile([128, 2, 512], BF16, tag="expt")
                    if r <= 0:
                        nc.scalar.activation(
                            out=expt.rearrange("p h q -> p (h q)"),
                            in_=sps.rearrange("p h q -> p (h q)"),
                            func=EXPF,
                            scale=SCEXP,
                        )
                    else:
                        nc.scalar.activation(
                            out=expt[:, :, soff:512],
                            in_=sps[:, :, soff:512],
                            func=EXPF,
                            scale=SCEXP,
                        )
                    if r >= 0:
                        # within-tile causal mask on the diagonal strip; the
                        # last diagonals gate the pair-end attnV drain, so
                        # run them on DVE (no Q7 launch latency)
                        tri_eng = nc.vector
                        tri_eng.tensor_tensor(
                            out=expt[:, :, soff : soff + 128],
                            in0=expt[:, :, soff : soff + 128],
                            in1=tri2_sb,
                            op=MUL,
                        )
                    lagged.append((expt, kb, r))
                    if len(lagged) > 6:
                        attnv(*lagged.popleft())
                    if kb == 1 and norm_q:
                        norm_q.popleft()()
                    if kb == 2:
                        alloc_acc()
                    if epi_q and kb == 6:
                        epi_q.popleft()()
                    if kb >= 8 and staged:
                        queue.extend(staged)
                        staged.clear()
                    pump()
                queue.extend(staged)
                staged.clear()
                drain_due(2 * qb + pair + 0.5)
                last = qb == NQB - 1 and pair == 1
                if last:
                    # h-major drain: head 0 finishes first so its normalize
                    # overlaps head 1's remaining matmuls
                    tail_kbs = list(lagged)
                    lagged.clear()
                else:
                    while lagged:
                        attnv(*lagged.popleft())
                while epi_q:
                    epi_q.popleft()()

                # normalize off the accumulators: batched reciprocal of the
                # ones-column denominators, then fused mult-evacuate to bf16.
                # Deferred into the next pair's kb==1 so the DVE chain never
                # sits at the PE queue head during the pair transition.
                attnq = pool_sm.tile([128, 4, 2, HD], BF16, tag="attnq")
                rec = pool_sm.tile([128, 2, 4], F32, tag="rec")

                def norm_h(h, accv=accv, attnq=attnq, rec=rec):
                    nc.vector.reciprocal(
                        out=rec[:, h, :],
                        in_=accv[h][:, :, HD : HD + 1].rearrange("p a c -> p (a c)"),
                    )
                    nc.vector.tensor_tensor(
                        out=attnq[:, :, h, :],
                        in0=accv[h][:, :, 0:HD],
                        in1=rec[:, h, :].broadcast_to([128, 4, HD]),
                        op=MUL,
                    )

                def norm_emit():
                    norm_h(0)
                    norm_h(1)

                ep = epilogue_tail(attnq, pair, qb)
                if last:
                    for h in range(2):
                        for expt_, kb_, r_ in tail_kbs:
                            for qt in range(max(r_, 0), 4):
                                nc.tensor.matmul(
                                    accv[h][:, qt, :],
                                    expt_[:, h, qt * 128 : qt * 128 + 128],
                                    vaug_sb[:, kb_, 2 * pair + h, :],
                                    start=False,
                                    stop=(kb_ == 4 * qb + qt),
                                    skip_group_check=True,
                                )
                        norm_h(h)
                    ep()
                else:
                    norm_q.append(norm_emit)
                    epi_q.append(ep)

            if qb == NQB - 1:
                for qs in range(4):
                    reserve.append(oproj_tail(qb * 512 + qs * 128))
            future_oproj -= 4 * 900

        attn_ctx.close()
        pp_tail = ctx.enter_context(
            tc.tile_pool(name="pp_tail", bufs=4, space="PSUM")
        )
        oproj_pool[0] = pp_tail
        while reserve:
            reserve.popleft()()
        while queue:
            _, _, emit = queue.popleft()
            emit()

    nc.compile()
    return nc


_NC = None


def _get_nc():
    global _NC
    if _NC is None:
        _NC = _build()
    return _NC


def kernel(x, mask, Wqkv, bqkv, Wout, bout):
    x = np.asarray(x, dtype=np.float32)
    Wqkv = np.asarray(Wqkv, dtype=np.float32)
    bqkv = np.asarray(bqkv, dtype=np.float32)
    Wout = np.asarray(Wout, dtype=np.float32)
    bout = np.asarray(bout, dtype=np.float32)
    assert not np.any(bqkv), "nonzero bqkv not supported by this kernel"

    import ml_dtypes

    bf16 = ml_dtypes.bfloat16
    f8 = ml_dtypes.float8_e4m3

    def hilo(a):
        hi = a.astype(f8)
        lo = (a - hi.astype(np.float32)).astype(f8)
        return np.ascontiguousarray(hi), np.ascontiguousarray(lo)

    # host-side layout prep; x and the qkv weights ship as fp8 hi/lo pairs
    xhis, xlos = [], []
    for b in range(B):
        xt = x[b].T.reshape(NDT, 128, S).transpose(1, 0, 2)  # [128, 8, 2048]
        hi, lo = hilo(xt)
        xhis.append(hi)
        xlos.append(lo)
    tri = np.triu(np.ones((128, 128), dtype=np.float32)).astype(bf16)
    tri2 = np.ascontiguousarray(np.stack([tri, tri], axis=1))  # [128, 2, 128]
    identv = np.ascontiguousarray(np.eye(128, dtype=np.float32).astype(bf16))

    def wslice(j, g):  # j: 0=q,1=k,2=v -> hi/lo [128, 8, 256] fp8
        cols = Wqkv[:, j * D + g * FL : j * D + (g + 1) * FL] * WSCALE
        wt = cols.reshape(NDT, 128, FL).transpose(1, 0, 2)
        return hilo(wt)

    in_maps = []
    for c in range(8):
        b, g = c // G, c % G
        whq_, wlq_ = wslice(0, g)
        whk_, wlk_ = wslice(1, g)
        whv_, wlv_ = wslice(2, g)
        wo = Wout[g * FL : (g + 1) * FL, :]  # [256, 1024]
        in_maps.append(
            {
                "xhi": xhis[b],
                "xlo": xlos[b],
                "whq": whq_,
                "wlq": wlq_,
                "whk": whk_,
                "wlk": wlk_,
                "whv": whv_,
                "wlv": wlv_,
                "wout": np.ascontiguousarray(
                    wo.reshape(2, 128, D).transpose(1, 0, 2).astype(bf16)
                ),
                "tri2": tri2,
                "ident": identv,
            }
        )

    nc = _get_nc()
    # axon terminals occasionally flake: transient NRT_EXEC_UNIT errors
    # (caught+retried) but also rare silent numeric corruption on a core.
    # Dispatch twice and cross-check; on mismatch, a third run breaks the
    # tie (device execution is deterministic, so good runs agree exactly).
    import time as _time

    def dispatch():
        for attempt in range(3):
            try:
                res = run_bass_kernel_spmd(nc, in_maps, core_ids=list(range(8)))
                break
            except Exception:
                if attempt == 2:
                    raise
                _time.sleep(2.0)
        out = np.empty((B, S, D), dtype=np.float32)
        for b in range(B):
            acc = res.results[b * G]["out_p"].astype(np.float32).copy()
            for g in range(1, G):
                acc += res.results[b * G + g]["out_p"]
            out[b] = acc + bout[None, :]
        return out

    def close(a, b):
        return np.linalg.norm(a - b) <= 1e-4 * np.linalg.norm(a)

    out1 = dispatch()
    out2 = dispatch()
    if close(out1, out2):
        return out1
    out3 = dispatch()
    if close(out1, out3):
        return out1
    if close(out2, out3):
        return out2
    return out3


# revision 11
# speedup vs baseline: 1.0158x; 1.0006x over previous
"""Causal self-attention on 8 trn2 NeuronCores — v2.

Sharding: core c = (b, g) with b = c // 4 (batch), g = c % 4 (head group of
4 heads).  Each core computes q/k/v projections for its 4 heads, causal
attention, and a partial out-projection (its 256 rows of Wout).  Host sums
the 4 partials per batch and adds bout.

v2 structural changes vs v1:
  * q/k/v projections run as fp8e4m3 DoubleRow matmuls with hi+lo error
    compensation (W' = 32*W split into Whi+Wlo, x into xhi+xlo; the three
    products Whi.xhi + Whi.xlo + Wlo.xhi land in one f32 psum).  25% fewer
    PE cycles than bf16 at ~bf16 accuracy; the 32x scale folds into the
    exp scale (q,k) and the v evacuation (x 1/32).
  * attnV swaps moving/stationary: expt tiles [128k x 128q] are the
    stationary operand, vaug [128k x 65] the moving one, accumulating into
    per-head psum accumulators [q, 4qt, 65] — 65-cycle matmuls instead of
    width-cycle ones (2x fewer PE cycles), with the softmax denominator in
    column 64 via the vaug ones-column.
  * normalization fuses into the accumulator evacuation (tensor_tensor with
    a stride-0-broadcast reciprocal), then PE transposes [q, f] -> [f, q]
    tiles through identity is_transpose matmuls for the out-projection.
  * psum accumulation uses one start=True per 2KB bank zero-region; sibling
    chains open start=False and rely on pending-zero (all psum tags are
    bank-sized so regions never straddle tiles).

Layouts on device:
  xhi/xlo  [128, 8, 2048] fp8   x[b]^T, d-tile major
  wh*/wl*  [128, 8, 256]  fp8   32*W columns for this group, d-tile major
  qT/kT    [128, 2, 2048] bf16  [2 heads x 64 hd][pair][pos], carries x32
  vaug     [128, 16, 4, 65] bf16  per k-tile, per head: 64 v-cols + ones
  expt     [128, 2, 512]  bf16  exp(scores^T) per k-tile, [k][head][q]
  attnT    [128, 2, 2048] bf16  normalized attn, features on partitions
"""

import sys

if "/opt/trn_rl_repo" not in sys.path:
    sys.path.insert(0, "/opt/trn_rl_repo")

import numpy as np

import concourse.mybir as mybir
import concourse.tile as tile
from concourse import bacc
from concourse.bass_utils import run_bass_kernel_spmd
from concourse.vector_clock import ScopedClock, VectorClock

B, S, D, H, HD = 2, 2048, 1024, 16, 64
G = 4            # head groups (cores per batch)
HL = H // G      # heads per core = 4
FL = HL * HD     # local features = 256
NQB = S // 512   # 4 q-blocks of 512
NST = S // 128   # 16 s-tiles of 128
NDT = D // 128   # 8 d-tiles
NDP = NDT // 2   # 4 d-tile pairs for DoubleRow

F32 = mybir.dt.float32
BF16 = mybir.dt.bfloat16
F8 = mybir.dt.float8e4
EXPF = mybir.ActivationFunctionType.Exp
DR = mybir.MatmulPerfMode.DoubleRow
MUL = mybir.AluOpType.mult

WSCALE = 32.0                    # W' = 32*W for fp8 hi/lo headroom
SCEXP = 0.125 / (WSCALE * WSCALE)  # exp scale: 1/sqrt(HD) / (32*32)


class SplitDrainTC(tile.TileContext):
    """This walrus build rejects >1 sync wait on an SP Drain; emit one
    drain per live proc instead of a single fat one."""

    def _drain_and_barrier(self, tick_clock, wait_clock):
        g = tick_clock.global_clock
        n = len(g)
        live = [(p, g[p]) for p in range(n) if g[p] > 0]
        if not live:
            self.nc.sync.drain()
        for p, t in live:
            vec = [0] * n
            vec[p] = t
            d = self.nc.sync.drain()
            wait_clock.add_sem_waits(d.ins, ScopedClock({None: VectorClock(vec)}))
        self.nc.all_engine_barrier()
        assert self.sems is not None
        popped = self.nc._tile_sem_poison_stack.pop()
        assert popped is self._sem_poison
        self.nc.clear_and_free_semaphores(list(self.sems.allocated().values()))
        self.nc.all_engine_barrier()


def _build(debug=False):
    nc = bacc.Bacc()
    xhi = nc.declare_dram_parameter("xhi", [128, NDT, S], F8, isOutput=False)
    xlo = nc.declare_dram_parameter("xlo", [128, NDT, S], F8, isOutput=False)
    whq = nc.declare_dram_parameter("whq", [128, NDT, FL], F8, isOutput=False)
    wlq = nc.declare_dram_parameter("wlq", [128, NDT, FL], F8, isOutput=False)
    whk = nc.declare_dram_parameter("whk", [128, NDT, FL], F8, isOutput=False)
    wlk = nc.declare_dram_parameter("wlk", [128, NDT, FL], F8, isOutput=False)
    whv = nc.declare_dram_parameter("whv", [128, NDT, FL], F8, isOutput=False)
    wlv = nc.declare_dram_parameter("wlv", [128, NDT, FL], F8, isOutput=False)
    wout = nc.declare_dram_parameter("wout", [128, 2, D], BF16, isOutput=False)
    tri2 = nc.declare_dram_parameter("tri2", [128, 2, 128], BF16, isOutput=False)
    ident = nc.declare_dram_parameter("ident", [128, 128], BF16, isOutput=False)
    out_p = nc.declare_dram_parameter("out_p", [S, D], BF16, isOutput=True)

    from collections import deque
    from contextlib import ExitStack

    with SplitDrainTC(nc) as tc, ExitStack() as ctx:
        consts = ctx.enter_context(tc.tile_pool(name="consts", bufs=1))
        pp_fill = ctx.enter_context(tc.tile_pool(name="pp_fill", bufs=2, space="PSUM"))
        attn_ctx = ExitStack()
        pp_sc = attn_ctx.enter_context(tc.tile_pool(name="pp_sc", bufs=2, space="PSUM"))
        pp_acc = attn_ctx.enter_context(
            tc.tile_pool(name="pp_acc", bufs=1, space="PSUM")
        )
        oproj_pool = [pp_fill]
        pool_exp = ctx.enter_context(tc.tile_pool(name="pool_exp", bufs=7))
        pool_out = ctx.enter_context(tc.tile_pool(name="pool_out", bufs=5))
        pool_sm = ctx.enter_context(tc.tile_pool(name="pool_sm", bufs=4))

        xhi_sb = consts.tile([128, NDT, S], F8)
        xlo_sb = consts.tile([128, NDT, S], F8)
        whq_sb = consts.tile([128, NDT, FL], F8)
        wlq_sb = consts.tile([128, NDT, FL], F8)
        whk_sb = consts.tile([128, NDT, FL], F8)
        wlk_sb = consts.tile([128, NDT, FL], F8)
        whv_sb = consts.tile([128, NDT, FL], F8)
        wlv_sb = consts.tile([128, NDT, FL], F8)
        wout_sb = consts.tile([128, 2, D], BF16)
        tri2_sb = consts.tile([128, 2, 128], BF16)
        id_sb = consts.tile([128, 128], BF16)
        qT_sb = consts.tile([128, 2, S], BF16)
        kT_sb = consts.tile([128, 2, S], BF16)
        vaug_sb = consts.tile([128, NST, HL, HD + 1], BF16)
        attnT_sb = consts.tile([128, 2, S], BF16)

        # PE clock-ramp warmup: dummy matmuls on zeroed SBUF while the
        # first DMAs land, so real matmuls start at full clock.
        nc.vector.memset(attnT_sb[:, 0, 0:256], 0.0)
        for i in range(64):
            wps = pp_fill.tile([128, 512], F32, tag="fill")
            nc.tensor.matmul(
                wps[:, 0:128],
                attnT_sb[:, 0, 0:128],
                attnT_sb[:, 0, 128:256],
                start=True,
                stop=True,
            )

        # DMA order matters: first matmuls need wq hi/lo and the first
        # s-block of xhi/xlo; weights issue from the (idle-at-start) ACT
        # queue so their descriptor generation runs parallel to the x
        # stream on SP.
        nc.scalar.dma_start(out=whq_sb, in_=whq[:])
        nc.scalar.dma_start(out=wlq_sb, in_=wlq[:])
        nc.sync.dma_start(out=xhi_sb[:, :, 0:512], in_=xhi[:, :, 0:512])
        nc.scalar.dma_start(out=whk_sb, in_=whk[:])
        nc.scalar.dma_start(out=wlk_sb, in_=wlk[:])
        nc.scalar.dma_start(out=tri2_sb, in_=tri2[:])
        nc.sync.dma_start(out=xlo_sb[:, :, 0:512], in_=xlo[:, :, 0:512])
        nc.scalar.dma_start(out=whv_sb, in_=whv[:])
        nc.scalar.dma_start(out=wlv_sb, in_=wlv[:])
        nc.scalar.dma_start(out=id_sb, in_=ident[:])
        nc.sync.dma_start(out=xhi_sb[:, :, 512:1024], in_=xhi[:, :, 512:1024])
        nc.sync.dma_start(out=xlo_sb[:, :, 512:1024], in_=xlo[:, :, 512:1024])
        nc.scalar.dma_start(out=wout_sb, in_=wout[:])
        nc.sync.dma_start(out=xhi_sb[:, :, 1024:S], in_=xhi[:, :, 1024:S])
        nc.sync.dma_start(out=xlo_sb[:, :, 1024:S], in_=xlo[:, :, 1024:S])
        # ACT spline-table preload for Exp, after the weight DMA issues so
        # it doesn't delay them on the ACT queue
        warm = pool_sm.tile([1, 1], F32, tag="warm")
        nc.vector.memset(warm, 0.0)
        nc.scalar.activation(out=warm, in_=warm, func=EXPF)
        # ones columns of vaug (constant across the run)
        nc.gpsimd.memset(vaug_sb[:, :, :, HD : HD + 1], 1.0)

        # ---- chunk emitters (projections / out-proj used as PE filler) ----
        def qkT_chunk(wh_sb, wl_sb, dst, ft, sb_):
            # 512 positions of one 128-feature column tile of q or k:
            # 2 pos-chunks x (4 d-pairs x 3 comp terms) DoubleRow matmuls.
            def emit():
                ps = pp_fill.tile([128, 512], F32, tag="fill")
                for c in range(2):
                    p0 = sb_ * 512 + c * 256
                    first = True
                    for dp in range(NDP):
                        dsl = slice(2 * dp, 2 * dp + 2)
                        fsl = slice(ft * 128, ft * 128 + 128)
                        for wmat, xmat in (
                            (wh_sb, xhi_sb),
                            (wl_sb, xhi_sb),
                            (wh_sb, xlo_sb),
                        ):
                            nc.tensor.matmul(
                                ps[:, c * 256 : c * 256 + 256],
                                wmat[:, dsl, fsl],
                                xmat[:, dsl, p0 : p0 + 256],
                                start=first,
                                stop=(dp == NDP - 1 and xmat is xlo_sb),
                                perf_mode=DR,
                                skip_group_check=True,
                            )
                            first = False
                nc.vector.tensor_copy(
                    out=dst[:, ft, sb_ * 512 : sb_ * 512 + 512], in_=ps[:, 0:512]
                )

            return emit

        def v_chunk(st):
            def emit():
                ps = pp_fill.tile([128, 512], F32, tag="fill")
                first = True
                for dp in range(NDP):
                    dsl = slice(2 * dp, 2 * dp + 2)
                    psl = slice(st * 128, st * 128 + 128)
                    for xmat, wmat in (
                        (xhi_sb, whv_sb),
                        (xlo_sb, whv_sb),
                        (xhi_sb, wlv_sb),
                    ):
                        nc.tensor.matmul(
                            ps[:, 0:FL],
                            xmat[:, dsl, psl],
                            wmat[:, dsl, :],
                            start=first,
                            stop=(dp == NDP - 1 and wmat is wlv_sb),
                            perf_mode=DR,
                            skip_group_check=True,
                        )
                        first = False
                # evacuate with the 1/32 descale (W' = 32*W)
                nc.vector.tensor_scalar(
                    out=vaug_sb[:, st, :, 0:HD],
                    in0=ps[:, 0:FL].rearrange("p (h e) -> p h e", h=HL),
                    scalar1=1.0 / WSCALE,
                    scalar2=None,
                    op0=MUL,
                )

            return emit

        def oproj_tail(q0):
            # tail variant: both 512-col halves of a q-tile, one combined
            # 2KB DMA; DVE and ACT each evacuate one half
            def emit():
                out_t = pool_out.tile([128, 2, 512], BF16, tag="outw")
                for dc in range(2):
                    ops = oproj_pool[0].tile([128, 512], F32, tag="fill")
                    for ft in range(2):
                        nc.tensor.matmul(
                            ops[:, 0:512],
                            attnT_sb[:, ft, q0 : q0 + 128],
                            wout_sb[:, ft, dc * 512 : dc * 512 + 512],
                            start=(ft == 0),
                            stop=(ft == 1),
                        )
                    if dc == 0:
                        nc.vector.tensor_copy(out=out_t[:, 0, :], in_=ops[:, 0:512])
                    else:
                        nc.scalar.copy(out=out_t[:, 1, :], in_=ops[:, 0:512])
                nc.sync.dma_start(
                    out=out_p[q0 : q0 + 128, :],
                    in_=out_t.rearrange("p a b -> p (a b)"),
                )

            return emit

        def oproj_half(q0, dc, late=False):
            def emit():
                ops = oproj_pool[0].tile([128, 512], F32, tag="fill")
                for ft in range(2):
                    nc.tensor.matmul(
                        ops[:, 0:512],
                        attnT_sb[:, ft, q0 : q0 + 128],
                        wout_sb[:, ft, dc * 512 : dc * 512 + 512],
                        start=(ft == 0),
                        stop=(ft == 1),
                    )
                out_t = pool_out.tile([128, 512], BF16, tag="out")
                if late and dc == 1:
                    # post-attention: ACT is idle, split the evacuations
                    nc.scalar.copy(out=out_t, in_=ops[:, 0:512])
                else:
                    nc.vector.tensor_copy(out=out_t, in_=ops[:, 0:512])
                nc.sync.dma_start(
                    out=out_p[q0 : q0 + 128, dc * 512 : dc * 512 + 512], in_=out_t
                )

            return emit

        # filler queue: (deadline, cost_ns, emit_fn); FIFO order respects deps.
        # deadline units: 2*qb + pair (+0.5 for "before this pair's attnV
        # drain"); drain_due forces everything due at each boundary.
        queue = deque()
        reserve = deque()
        for qb in range(NQB):
            for wh_sb, wl_sb, dst in (
                (whq_sb, wlq_sb, qT_sb),
                (whk_sb, wlk_sb, kT_sb),
            ):
                if qb > 0:
                    queue.append(
                        (2 * qb - 1.25, 1300, qkT_chunk(wh_sb, wl_sb, dst, 0, qb))
                    )
            for st in range(4 * qb, 4 * qb + 4):
                queue.append((2 * qb + 0.5, 650, v_chunk(st)))
            for wh_sb, wl_sb, dst in (
                (whq_sb, wlq_sb, qT_sb),
                (whk_sb, wlk_sb, kT_sb),
            ):
                queue.append(
                    (
                        max(0.75, 2 * qb - 0.25),
                        1300,
                        qkT_chunk(wh_sb, wl_sb, dst, 1, qb),
                    )
                )

        # Adaptive pump: spread remaining filler cost over remaining attention
        # steps so late q-blocks (which have no projections left) still get
        # out-proj chunks as PE filler.
        total_steps = sum(2 * (4 * qb + 4) for qb in range(NQB))  # 80
        future_oproj = 4 * NQB * 900
        step_no = 0

        tokens = 0.0
        PUMP_RATE = 355.0  # ~per-step PE deficit vs the ACT exp stream

        def pump():
            nonlocal step_no, future_oproj, tokens
            step_no += 1
            tokens += PUMP_RATE
            while queue and tokens >= queue[0][1]:
                _, cost, emit = queue.popleft()
                emit()
                tokens -= cost

        def drain_due(qb):
            while queue and queue[0][0] <= qb:
                _, _, emit = queue.popleft()
                emit()

        # ---- prologue: only what (qb0, pair0) scores need; the rest
        # streams in as filler during pair0 ----
        qkT_chunk(whq_sb, wlq_sb, qT_sb, 0, 0)()
        qkT_chunk(whk_sb, wlk_sb, kT_sb, 0, 0)()

        # deferred per-(qb,pair) epilogue (transposes + attnT evac), emitted
        # a few kb-steps into the NEXT pair so PE never waits on the DVE
        # normalize chain
        epi_q = deque()
        norm_q = deque()
        staged = deque()

        def epilogue_tail(attnq, pair_, qb_):
            def emit():
                tp = pp_fill.tile([128, 512], F32, tag="fill")
                tpb = tp.bitcast(BF16)
                for qt in range(4):
                    nc.tensor.matmul(
                        tpb[:, qt * 128 : qt * 128 + 128],
                        attnq[:, qt, :, :].rearrange("p h f -> p (h f)"),
                        id_sb,
                        start=(qt == 0),
                        stop=(qt == 3),
                        is_transpose=True,
                        skip_group_check=True,
                    )
                nc.vector.tensor_copy(
                    out=attnT_sb[:, pair_, qb_ * 512 : qb_ * 512 + 512],
                    in_=tpb[:, 0:512],
                )
                if pair_ == 1:
                    # attnT for qb_ is complete -> its out-proj becomes
                    # filler, but hold it a few kb-steps so the pump can't
                    # pop it while the attnT evacuation is still in flight.
                    for qs_ in range(4):
                        for dc_ in range(2):
                            staged.append(
                                (
                                    100,
                                    450,
                                    oproj_half(
                                        qb_ * 512 + qs_ * 128,
                                        dc_,
                                        late=(qb_ == NQB - 1),
                                    ),
                                )
                            )

            return emit

        # ---- attention (scores -> exp/mask -> lagged swapped attnV) ----
        for qb in range(NQB):
            for pair in range(2):
                drain_due(2 * qb + pair)
                nkb = 4 * qb + 4
                # acc tiles are allocated lazily at kb==2, after the previous
                # pair's deferred normalize has been emitted (pool WAR
                # tracking needs readers emitted before the next allocation)
                accv = [None, None]

                def alloc_acc(accv=accv):
                    for h in range(2):
                        a = pp_acc.tile([128, 512], F32, tag=f"acc{h}")
                        accv[h] = a[:, 0:260].rearrange("p (a c) -> p a c", c=HD + 1)

                lagged = deque()  # expt tiles awaiting their attnV matmuls

                def attnv(expt, kb, r, accv=accv, pair=pair, qb=qb):
                    for qt in range(max(r, 0), 4):
                        for h in range(2):
                            nc.tensor.matmul(
                                accv[h][:, qt, :],
                                expt[:, h, qt * 128 : qt * 128 + 128],
                                vaug_sb[:, kb, 2 * pair + h, :],
                                start=(kb == 0 and qt == max(r, 0)),
                                stop=(kb == 4 * qb + qt),
                                skip_group_check=True,
                            )

                for kb in range(nkb):
                    r = kb - 4 * qb
                    soff = 128 * max(r, 0)
                    sps = pp_sc.tile([128, 2, 512], F32, tag="ps")
                    for h in range(2):
                        hp = slice(64 * h, 64 * h + 64)
                        nc.tensor.matmul(
                            sps[:, h, soff:512],
                            kT_sb[hp, pair, kb * 128 : kb * 128 + 128],
                            qT_sb[hp, pair, qb * 512 + soff : qb * 512 + 512],
                            start=True,
                            stop=True,
                        )
                    expt = pool_exp.tile([128, 2, 512], BF16, tag="expt")
                    if r <= 0:
                        nc.scalar.activation(
                            out=expt.rearrange("p h q -> p (h q)"),
                            in_=sps.rearrange("p h q -> p (h q)"),
                            func=EXPF,
                            scale=SCEXP,
                        )
                    else:
                        nc.scalar.activation(
                            out=expt[:, :, soff:512],
                            in_=sps[:, :, soff:512],
                            func=EXPF,
                            scale=SCEXP,
                        )
                    if r >= 0:
                        # within-tile causal mask on the diagonal strip; the
                        # last diagonals gate the pair-end attnV drain, so
                        # run them on DVE (no Q7 launch latency)
                        tri_eng = nc.vector
                        tri_eng.tensor_tensor(
                            out=expt[:, :, soff : soff + 128],
                            in0=expt[:, :, soff : soff + 128],
                            in1=tri2_sb,
                            op=MUL,
                        )
                    lagged.append((expt, kb, r))
                    if len(lagged) > 6:
                        attnv(*lagged.popleft())
                    if kb == 2 and norm_q:
                        norm_q.popleft()()
                    if kb == 3:
                        alloc_acc()
                    if epi_q and kb == 6:
                        epi_q.popleft()()
                    if kb >= 8 and staged:
                        queue.extend(staged)
                        staged.clear()
                    pump()
                queue.extend(staged)
                staged.clear()
                drain_due(2 * qb + pair + 0.5)
                last = qb == NQB - 1 and pair == 1
                if last:
                    # h-major drain: head 0 finishes first so its normalize
                    # overlaps head 1's remaining matmuls
                    tail_kbs = list(lagged)
                    lagged.clear()
                else:
                    while lagged:
                        attnv(*lagged.popleft())
                while epi_q:
                    epi_q.popleft()()

                # normalize off the accumulators: batched reciprocal of the
                # ones-column denominators, then fused mult-evacuate to bf16.
                # Deferred into the next pair's kb==1 so the DVE chain never
                # sits at the PE queue head during the pair transition.
                attnq = pool_sm.tile([128, 4, 2, HD], BF16, tag="attnq")
                rec = pool_sm.tile([128, 2, 4], F32, tag="rec")

                def norm_h(h, accv=accv, attnq=attnq, rec=rec):
                    nc.vector.reciprocal(
                        out=rec[:, h, :],
                        in_=accv[h][:, :, HD : HD + 1].rearrange("p a c -> p (a c)"),
                    )
                    nc.vector.tensor_tensor(
                        out=attnq[:, :, h, :],
                        in0=accv[h][:, :, 0:HD],
                        in1=rec[:, h, :].broadcast_to([128, 4, HD]),
                        op=MUL,
                    )

                def norm_emit():
                    norm_h(0)
                    norm_h(1)

                ep = epilogue_tail(attnq, pair, qb)
                if last:
                    for h in range(2):
                        for expt_, kb_, r_ in tail_kbs:
                            for qt in range(max(r_, 0), 4):
                                nc.tensor.matmul(
                                    accv[h][:, qt, :],
                                    expt_[:, h, qt * 128 : qt * 128 + 128],
                                    vaug_sb[:, kb_, 2 * pair + h, :],
                                    start=False,
                                    stop=(kb_ == 4 * qb + qt),
                                    skip_group_check=True,
                                )
                        norm_h(h)
                    ep()
                else:
                    norm_q.append(norm_emit)
                    epi_q.append(ep)

            if qb == NQB - 1:
                for qs in range(4):
                    reserve.append(oproj_tail(qb * 512 + qs * 128))
            future_oproj -= 4 * 900

        attn_ctx.close()
        pp_tail = ctx.enter_context(
            tc.tile_pool(name="pp_tail", bufs=4, space="PSUM")
        )
        oproj_pool[0] = pp_tail
        while reserve:
            reserve.popleft()()
        while queue:
            _, _, emit = queue.popleft()
            emit()

    nc.compile()
    return nc


_NC = None


def _get_nc():
    global _NC
    if _NC is None:
        _NC = _build()
    return _NC


def kernel(x, mask, Wqkv, bqkv, Wout, bout):
    x = np.asarray(x, dtype=np.float32)
    Wqkv = np.asarray(Wqkv, dtype=np.float32)
    bqkv = np.asarray(bqkv, dtype=np.float32)
    Wout = np.asarray(Wout, dtype=np.float32)
    bout = np.asarray(bout, dtype=np.float32)
    assert not np.any(bqkv), "nonzero bqkv not supported by this kernel"

    import ml_dtypes

    bf16 = ml_dtypes.bfloat16
    f8 = ml_dtypes.float8_e4m3

    def hilo(a):
        hi = a.astype(f8)
        lo = (a - hi.astype(np.float32)).astype(f8)
        return np.ascontiguousarray(hi), np.ascontiguousarray(lo)

    # host-side layout prep; x and the qkv weights ship as fp8 hi/lo pairs
    xhis, xlos = [], []
    for b in range(B):
        xt = x[b].T.reshape(NDT, 128, S).transpose(1, 0, 2)  # [128, 8, 2048]
        hi, lo = hilo(xt)
        xhis.append(hi)
        xlos.append(lo)
    tri = np.triu(np.ones((128, 128), dtype=np.float32)).astype(bf16)
    tri2 = np.ascontiguousarray(np.stack([tri, tri], axis=1))  # [128, 2, 128]
    identv = np.ascontiguousarray(np.eye(128, dtype=np.float32).astype(bf16))

    def wslice(j, g):  # j: 0=q,1=k,2=v -> hi/lo [128, 8, 256] fp8
        cols = Wqkv[:, j * D + g * FL : j * D + (g + 1) * FL] * WSCALE
        wt = cols.reshape(NDT, 128, FL).transpose(1, 0, 2)
        return hilo(wt)

    in_maps = []
    for c in range(8):
        b, g = c // G, c % G
        whq_, wlq_ = wslice(0, g)
        whk_, wlk_ = wslice(1, g)
        whv_, wlv_ = wslice(2, g)
        wo = Wout[g * FL : (g + 1) * FL, :]  # [256, 1024]
        in_maps.append(
            {
                "xhi": xhis[b],
                "xlo": xlos[b],
                "whq": whq_,
                "wlq": wlq_,
                "whk": whk_,
                "wlk": wlk_,
                "whv": whv_,
                "wlv": wlv_,
                "wout": np.ascontiguousarray(
                    wo.reshape(2, 128, D).transpose(1, 0, 2).astype(bf16)
                ),
                "tri2": tri2,
                "ident": identv,
            }
        )

    nc = _get_nc()
    # axon terminals occasionally flake: transient NRT_EXEC_UNIT errors
    # (caught+retried) but also rare silent numeric corruption on a core.
    # Dispatch twice and cross-check; on mismatch, a third run breaks the
    # tie (device execution is deterministic, so good runs agree exactly).
    import time as _time

    def dispatch():
        for attempt in range(3):
            try:
                res = run_bass_kernel_spmd(nc, in_maps, core_ids=list(range(8)))
                break
            except Exception:
                if attempt == 2:
                    raise
                _time.sleep(2.0)
        out = np.empty((B, S, D), dtype=np.float32)
        for b in range(B):
            acc = res.results[b * G]["out_p"].astype(np.float32).copy()
            for g in range(1, G):
                acc += res.results[b * G + g]["out_p"]
            out[b] = acc + bout[None, :]
        return out

    def close(a, b):
        return np.linalg.norm(a - b) <= 1e-4 * np.linalg.norm(a)

    out1 = dispatch()
    out2 = dispatch()
    if close(out1, out2):
        return out1
    out3 = dispatch()
    if close(out1, out3):
        return out1
    if close(out2, out3):
        return out2
    return out3


# revision 12
# speedup vs baseline: 1.0256x; 1.0096x over previous
"""Causal self-attention on 8 trn2 NeuronCores — v2.

Sharding: core c = (b, g) with b = c // 4 (batch), g = c % 4 (head group of
4 heads).  Each core computes q/k/v projections for its 4 heads, causal
attention, and a partial out-projection (its 256 rows of Wout).  Host sums
the 4 partials per batch and adds bout.

v2 structural changes vs v1:
  * q/k/v projections run as fp8e4m3 DoubleRow matmuls with hi+lo error
    compensation (W' = 32*W split into Whi+Wlo, x into xhi+xlo; the three
    products Whi.xhi + Whi.xlo + Wlo.xhi land in one f32 psum).  25% fewer
    PE cycles than bf16 at ~bf16 accuracy; the 32x scale folds into the
    exp scale (q,k) and the v evacuation (x 1/32).
  * attnV swaps moving/stationary: expt tiles [128k x 128q] are the
    stationary operand, vaug [128k x 65] the moving one, accumulating into
    per-head psum accumulators [q, 4qt, 65] — 65-cycle matmuls instead of
    width-cycle ones (2x fewer PE cycles), with the softmax denominator in
    column 64 via the vaug ones-column.
  * normalization fuses into the accumulator evacuation (tensor_tensor with
    a stride-0-broadcast reciprocal), then PE transposes [q, f] -> [f, q]
    tiles through identity is_transpose matmuls for the out-projection.
  * psum accumulation uses one start=True per 2KB bank zero-region; sibling
    chains open start=False and rely on pending-zero (all psum tags are
    bank-sized so regions never straddle tiles).

Layouts on device:
  xhi/xlo  [128, 8, 2048] fp8   x[b]^T, d-tile major
  wh*/wl*  [128, 8, 256]  fp8   32*W columns for this group, d-tile major
  qT/kT    [128, 2, 2048] bf16  [2 heads x 64 hd][pair][pos], carries x32
  vaug     [128, 16, 4, 65] bf16  per k-tile, per head: 64 v-cols + ones
  expt     [128, 2, 512]  bf16  exp(scores^T) per k-tile, [k][head][q]
  attnT    [128, 2, 2048] bf16  normalized attn, features on partitions
"""

import sys

if "/opt/trn_rl_repo" not in sys.path:
    sys.path.insert(0, "/opt/trn_rl_repo")

import numpy as np

import concourse.mybir as mybir
import concourse.tile as tile
from concourse import bacc
from concourse.bass_utils import run_bass_kernel_spmd
from concourse.vector_clock import ScopedClock, VectorClock

B, S, D, H, HD = 2, 2048, 1024, 16, 64
G = 4            # head groups (cores per batch)
HL = H // G      # heads per core = 4
FL = HL * HD     # local features = 256
NQB = S // 512   # 4 q-blocks of 512
NST = S // 128   # 16 s-tiles of 128
NDT = D // 128   # 8 d-tiles
NDP = NDT // 2   # 4 d-tile pairs for DoubleRow

F32 = mybir.dt.float32
BF16 = mybir.dt.bfloat16
F8 = mybir.dt.float8e4
EXPF = mybir.ActivationFunctionType.Exp
DR = mybir.MatmulPerfMode.DoubleRow
MUL = mybir.AluOpType.mult

WSCALE = 32.0                    # W' = 32*W for fp8 hi/lo headroom
SCEXP = 0.125 / (WSCALE * WSCALE)  # exp scale: 1/sqrt(HD) / (32*32)


class SplitDrainTC(tile.TileContext):
    """This walrus build rejects >1 sync wait on an SP Drain; emit one
    drain per live proc instead of a single fat one."""

    def _drain_and_barrier(self, tick_clock, wait_clock):
        g = tick_clock.global_clock
        n = len(g)
        live = [(p, g[p]) for p in range(n) if g[p] > 0]
        if not live:
            self.nc.sync.drain()
        for p, t in live:
            vec = [0] * n
            vec[p] = t
            d = self.nc.sync.drain()
            wait_clock.add_sem_waits(d.ins, ScopedClock({None: VectorClock(vec)}))
        self.nc.all_engine_barrier()
        assert self.sems is not None
        popped = self.nc._tile_sem_poison_stack.pop()
        assert popped is self._sem_poison
        self.nc.clear_and_free_semaphores(list(self.sems.allocated().values()))
        self.nc.all_engine_barrier()


def _build(debug=False):
    nc = bacc.Bacc()
    xhi = nc.declare_dram_parameter("xhi", [128, NDT, S], F8, isOutput=False)
    xlo = nc.declare_dram_parameter("xlo", [128, NDT, S], F8, isOutput=False)
    whq = nc.declare_dram_parameter("whq", [128, NDT, FL], F8, isOutput=False)
    wlq = nc.declare_dram_parameter("wlq", [128, NDT, FL], F8, isOutput=False)
    whk = nc.declare_dram_parameter("whk", [128, NDT, FL], F8, isOutput=False)
    wlk = nc.declare_dram_parameter("wlk", [128, NDT, FL], F8, isOutput=False)
    whv = nc.declare_dram_parameter("whv", [128, NDT, FL], F8, isOutput=False)
    wlv = nc.declare_dram_parameter("wlv", [128, NDT, FL], F8, isOutput=False)
    wout = nc.declare_dram_parameter("wout", [128, 2, D], BF16, isOutput=False)
    tri2 = nc.declare_dram_parameter("tri2", [128, 2, 128], BF16, isOutput=False)
    ident = nc.declare_dram_parameter("ident", [128, 128], BF16, isOutput=False)
    out_p = nc.declare_dram_parameter("out_p", [S, D], BF16, isOutput=True)

    from collections import deque
    from contextlib import ExitStack

    with SplitDrainTC(nc) as tc, ExitStack() as ctx:
        consts = ctx.enter_context(tc.tile_pool(name="consts", bufs=1))
        pp_fill = ctx.enter_context(tc.tile_pool(name="pp_fill", bufs=2, space="PSUM"))
        attn_ctx = ExitStack()
        pp_sc = attn_ctx.enter_context(tc.tile_pool(name="pp_sc", bufs=2, space="PSUM"))
        pp_acc = attn_ctx.enter_context(
            tc.tile_pool(name="pp_acc", bufs=1, space="PSUM")
        )
        oproj_pool = [pp_fill]
        pool_exp = ctx.enter_context(tc.tile_pool(name="pool_exp", bufs=7))
        pool_out = ctx.enter_context(tc.tile_pool(name="pool_out", bufs=5))
        pool_sm = ctx.enter_context(tc.tile_pool(name="pool_sm", bufs=4))

        xhi_sb = consts.tile([128, NDT, S], F8)
        xlo_sb = consts.tile([128, NDT, S], F8)
        whq_sb = consts.tile([128, NDT, FL], F8)
        wlq_sb = consts.tile([128, NDT, FL], F8)
        whk_sb = consts.tile([128, NDT, FL], F8)
        wlk_sb = consts.tile([128, NDT, FL], F8)
        whv_sb = consts.tile([128, NDT, FL], F8)
        wlv_sb = consts.tile([128, NDT, FL], F8)
        wout_sb = consts.tile([128, 2, D], BF16)
        tri2_sb = consts.tile([128, 2, 128], BF16)
        id_sb = consts.tile([128, 128], BF16)
        qT_sb = consts.tile([128, 2, S], BF16)
        kT_sb = consts.tile([128, 2, S], BF16)
        vaug_sb = consts.tile([128, NST, HL, HD + 1], BF16)
        attnT_sb = consts.tile([128, 2, S], BF16)

        # PE clock-ramp warmup: dummy matmuls on zeroed SBUF while the
        # first DMAs land, so real matmuls start at full clock.
        nc.vector.memset(attnT_sb[:, 0, 0:256], 0.0)
        for i in range(64):
            wps = pp_fill.tile([128, 512], F32, tag="fill")
            nc.tensor.matmul(
                wps[:, 0:128],
                attnT_sb[:, 0, 0:128],
                attnT_sb[:, 0, 128:256],
                start=True,
                stop=True,
            )

        # DMA order matters: first matmuls need wq hi/lo and the first
        # s-block of xhi/xlo; weights issue from the (idle-at-start) ACT
        # queue so their descriptor generation runs parallel to the x
        # stream on SP.
        nc.scalar.dma_start(out=whq_sb, in_=whq[:])
        nc.scalar.dma_start(out=wlq_sb, in_=wlq[:])
        nc.sync.dma_start(out=xhi_sb[:, :, 0:512], in_=xhi[:, :, 0:512])
        nc.scalar.dma_start(out=whk_sb, in_=whk[:])
        nc.scalar.dma_start(out=wlk_sb, in_=wlk[:])
        nc.scalar.dma_start(out=tri2_sb, in_=tri2[:])
        nc.sync.dma_start(out=xlo_sb[:, :, 0:512], in_=xlo[:, :, 0:512])
        nc.scalar.dma_start(out=whv_sb, in_=whv[:])
        nc.scalar.dma_start(out=wlv_sb, in_=wlv[:])
        nc.scalar.dma_start(out=id_sb, in_=ident[:])
        nc.sync.dma_start(out=xhi_sb[:, :, 512:1024], in_=xhi[:, :, 512:1024])
        nc.sync.dma_start(out=xlo_sb[:, :, 512:1024], in_=xlo[:, :, 512:1024])
        nc.scalar.dma_start(out=wout_sb, in_=wout[:])
        nc.sync.dma_start(out=xhi_sb[:, :, 1024:S], in_=xhi[:, :, 1024:S])
        nc.sync.dma_start(out=xlo_sb[:, :, 1024:S], in_=xlo[:, :, 1024:S])
        # ACT spline-table preload for Exp, after the weight DMA issues so
        # it doesn't delay them on the ACT queue
        warm = pool_sm.tile([1, 1], F32, tag="warm")
        nc.vector.memset(warm, 0.0)
        nc.scalar.activation(out=warm, in_=warm, func=EXPF)
        # ones columns of vaug (constant across the run)
        nc.gpsimd.memset(vaug_sb[:, :, :, HD : HD + 1], 1.0)

        # ---- chunk emitters (projections / out-proj used as PE filler) ----
        def qkT_chunk(wh_sb, wl_sb, dst, ft, sb_):
            # 512 positions of one 128-feature column tile of q or k:
            # 2 pos-chunks x (4 d-pairs x 3 comp terms) DoubleRow matmuls.
            def emit():
                ps = pp_fill.tile([128, 512], F32, tag="fill")
                for c in range(2):
                    p0 = sb_ * 512 + c * 256
                    first = True
                    for dp in range(NDP):
                        dsl = slice(2 * dp, 2 * dp + 2)
                        fsl = slice(ft * 128, ft * 128 + 128)
                        for wmat, xmat in (
                            (wh_sb, xhi_sb),
                            (wl_sb, xhi_sb),
                            (wh_sb, xlo_sb),
                        ):
                            nc.tensor.matmul(
                                ps[:, c * 256 : c * 256 + 256],
                                wmat[:, dsl, fsl],
                                xmat[:, dsl, p0 : p0 + 256],
                                start=first,
                                stop=(dp == NDP - 1 and xmat is xlo_sb),
                                perf_mode=DR,
                                skip_group_check=True,
                            )
                            first = False
                nc.vector.tensor_copy(
                    out=dst[:, ft, sb_ * 512 : sb_ * 512 + 512], in_=ps[:, 0:512]
                )

            return emit

        def v_chunk(st):
            def emit():
                ps = pp_fill.tile([128, 512], F32, tag="fill")
                first = True
                for dp in range(NDP):
                    dsl = slice(2 * dp, 2 * dp + 2)
                    psl = slice(st * 128, st * 128 + 128)
                    for xmat, wmat in (
                        (xhi_sb, whv_sb),
                        (xlo_sb, whv_sb),
                        (xhi_sb, wlv_sb),
                    ):
                        nc.tensor.matmul(
                            ps[:, 0:FL],
                            xmat[:, dsl, psl],
                            wmat[:, dsl, :],
                            start=first,
                            stop=(dp == NDP - 1 and wmat is wlv_sb),
                            perf_mode=DR,
                            skip_group_check=True,
                        )
                        first = False
                # evacuate with the 1/32 descale (W' = 32*W)
                nc.vector.tensor_scalar(
                    out=vaug_sb[:, st, :, 0:HD],
                    in0=ps[:, 0:FL].rearrange("p (h e) -> p h e", h=HL),
                    scalar1=1.0 / WSCALE,
                    scalar2=None,
                    op0=MUL,
                )

            return emit

        def oproj_tail(q0):
            # tail variant: both 512-col halves of a q-tile, one combined
            # 2KB DMA; DVE and ACT each evacuate one half
            def emit():
                out_t = pool_out.tile([128, 2, 512], BF16, tag="outw")
                for dc in range(2):
                    ops = oproj_pool[0].tile([128, 512], F32, tag="fill")
                    for ft in range(2):
                        nc.tensor.matmul(
                            ops[:, 0:512],
                            attnT_sb[:, ft, q0 : q0 + 128],
                            wout_sb[:, ft, dc * 512 : dc * 512 + 512],
                            start=(ft == 0),
                            stop=(ft == 1),
                        )
                    if dc == 0:
                        nc.vector.tensor_copy(out=out_t[:, 0, :], in_=ops[:, 0:512])
                    else:
                        nc.scalar.copy(out=out_t[:, 1, :], in_=ops[:, 0:512])
                nc.sync.dma_start(
                    out=out_p[q0 : q0 + 128, :],
                    in_=out_t.rearrange("p a b -> p (a b)"),
                )

            return emit

        def oproj_half(q0, dc, late=False):
            def emit():
                ops = oproj_pool[0].tile([128, 512], F32, tag="fill")
                for ft in range(2):
                    nc.tensor.matmul(
                        ops[:, 0:512],
                        attnT_sb[:, ft, q0 : q0 + 128],
                        wout_sb[:, ft, dc * 512 : dc * 512 + 512],
                        start=(ft == 0),
                        stop=(ft == 1),
                    )
                out_t = pool_out.tile([128, 512], BF16, tag="out")
                if late and dc == 1:
                    # post-attention: ACT is idle, split the evacuations
                    nc.scalar.copy(out=out_t, in_=ops[:, 0:512])
                else:
                    nc.vector.tensor_copy(out=out_t, in_=ops[:, 0:512])
                nc.sync.dma_start(
                    out=out_p[q0 : q0 + 128, dc * 512 : dc * 512 + 512], in_=out_t
                )

            return emit

        # filler queue: (deadline, cost_ns, emit_fn); FIFO order respects deps.
        # deadline units: 2*qb + pair (+0.5 for "before this pair's attnV
        # drain"); drain_due forces everything due at each boundary.
        queue = deque()
        reserve = deque()
        for qb in range(NQB):
            # two v-chunks lead each qb segment so the pump interleaves the
            # cheap filler finely; their deadline stays late so they are
            # pump-paced rather than burst at the forced-drain points (the
            # build-time pump provably emits the qk chunks behind them before
            # their consumers - verified by the hardware numerics check)
            for st in range(4 * qb, 4 * qb + 2):
                queue.append((2 * qb + 0.5, 650, v_chunk(st)))
            for wh_sb, wl_sb, dst in (
                (whq_sb, wlq_sb, qT_sb),
                (whk_sb, wlk_sb, kT_sb),
            ):
                if qb > 0:
                    queue.append(
                        (2 * qb - 1.25, 1300, qkT_chunk(wh_sb, wl_sb, dst, 0, qb))
                    )
            for st in range(4 * qb + 2, 4 * qb + 4):
                queue.append((2 * qb + 0.5, 650, v_chunk(st)))
            for wh_sb, wl_sb, dst in (
                (whq_sb, wlq_sb, qT_sb),
                (whk_sb, wlk_sb, kT_sb),
            ):
                queue.append(
                    (
                        max(0.75, 2 * qb - 0.25),
                        1300,
                        qkT_chunk(wh_sb, wl_sb, dst, 1, qb),
                    )
                )

        # Adaptive pump: spread remaining filler cost over remaining attention
        # steps so late q-blocks (which have no projections left) still get
        # out-proj chunks as PE filler.
        total_steps = sum(2 * (4 * qb + 4) for qb in range(NQB))  # 80
        future_oproj = 4 * NQB * 900
        step_no = 0

        tokens = 0.0
        PUMP_RATE = 355.0  # ~per-step PE deficit vs the ACT exp stream

        def pump():
            nonlocal step_no, future_oproj, tokens
            step_no += 1
            tokens += PUMP_RATE
            while queue and tokens >= queue[0][1]:
                _, cost, emit = queue.popleft()
                emit()
                tokens -= cost

        def drain_due(qb):
            while queue and queue[0][0] <= qb:
                _, _, emit = queue.popleft()
                emit()

        # ---- prologue: only what (qb0, pair0) scores need; the rest
        # streams in as filler during pair0 ----
        qkT_chunk(whq_sb, wlq_sb, qT_sb, 0, 0)()
        qkT_chunk(whk_sb, wlk_sb, kT_sb, 0, 0)()

        # deferred per-(qb,pair) epilogue (transposes + attnT evac), emitted
        # a few kb-steps into the NEXT pair so PE never waits on the DVE
        # normalize chain
        epi_q = deque()
        norm_q = deque()
        staged = deque()

        def epilogue_tail(attnq, pair_, qb_):
            def emit():
                tp = pp_fill.tile([128, 512], F32, tag="fill")
                tpb = tp.bitcast(BF16)
                for qt in range(4):
                    nc.tensor.matmul(
                        tpb[:, qt * 128 : qt * 128 + 128],
                        attnq[:, qt, :, :].rearrange("p h f -> p (h f)"),
                        id_sb,
                        start=(qt == 0),
                        stop=(qt == 3),
                        is_transpose=True,
                        skip_group_check=True,
                    )
                nc.vector.tensor_copy(
                    out=attnT_sb[:, pair_, qb_ * 512 : qb_ * 512 + 512],
                    in_=tpb[:, 0:512],
                )
                if pair_ == 1:
                    # attnT for qb_ is complete -> its out-proj becomes
                    # filler, but hold it a few kb-steps so the pump can't
                    # pop it while the attnT evacuation is still in flight.
                    for qs_ in range(4):
                        for dc_ in range(2):
                            staged.append(
                                (
                                    100,
                                    450,
                                    oproj_half(
                                        qb_ * 512 + qs_ * 128,
                                        dc_,
                                        late=(qb_ == NQB - 1),
                                    ),
                                )
                            )

            return emit

        # ---- attention (scores -> exp/mask -> lagged swapped attnV) ----
        for qb in range(NQB):
            for pair in range(2):
                drain_due(2 * qb + pair)
                nkb = 4 * qb + 4
                # acc tiles are allocated lazily at kb==2, after the previous
                # pair's deferred normalize has been emitted (pool WAR
                # tracking needs readers emitted before the next allocation)
                accv = [None, None]

                def alloc_acc(accv=accv):
                    for h in range(2):
                        a = pp_acc.tile([128, 512], F32, tag=f"acc{h}")
                        accv[h] = a[:, 0:260].rearrange("p (a c) -> p a c", c=HD + 1)

                lagged = deque()  # expt tiles awaiting their attnV matmuls

                def attnv(expt, kb, r, accv=accv, pair=pair, qb=qb):
                    for qt in range(max(r, 0), 4):
                        for h in range(2):
                            nc.tensor.matmul(
                                accv[h][:, qt, :],
                                expt[:, h, qt * 128 : qt * 128 + 128],
                                vaug_sb[:, kb, 2 * pair + h, :],
                                start=(kb == 0 and qt == max(r, 0)),
                                stop=(kb == 4 * qb + qt),
                                skip_group_check=True,
                            )

                for kb in range(nkb):
                    r = kb - 4 * qb
                    soff = 128 * max(r, 0)
                    sps = pp_sc.tile([128, 2, 512], F32, tag="ps")
                    for h in range(2):
                        hp = slice(64 * h, 64 * h + 64)
                        nc.tensor.matmul(
                            sps[:, h, soff:512],
                            kT_sb[hp, pair, kb * 128 : kb * 128 + 128],
                            qT_sb[hp, pair, qb * 512 + soff : qb * 512 + 512],
                            start=True,
                            stop=True,
                        )
                    expt = pool_exp.tile([128, 2, 512], BF16, tag="expt")
                    if r <= 0:
                        nc.scalar.activation(
                            out=expt.rearrange("p h q -> p (h q)"),
                            in_=sps.rearrange("p h q -> p (h q)"),
                            func=EXPF,
                            scale=SCEXP,
                        )
                    else:
                        nc.scalar.activation(
                            out=expt[:, :, soff:512],
                            in_=sps[:, :, soff:512],
                            func=EXPF,
                            scale=SCEXP,
                        )
                    if r >= 0:
                        # within-tile causal mask on the diagonal strip; the
                        # last diagonals gate the pair-end attnV drain, so
                        # run them on DVE (no Q7 launch latency)
                        tri_eng = nc.vector
                        tri_eng.tensor_tensor(
                            out=expt[:, :, soff : soff + 128],
                            in0=expt[:, :, soff : soff + 128],
                            in1=tri2_sb,
                            op=MUL,
                        )
                    lagged.append((expt, kb, r))
                    if len(lagged) > 6:
                        attnv(*lagged.popleft())
                    if kb == 2 and norm_q:
                        norm_q.popleft()()
                    if kb == 3:
                        alloc_acc()
                    if epi_q and kb == 6:
                        epi_q.popleft()()
                    if kb >= 8 and staged:
                        queue.extend(staged)
                        staged.clear()
                    pump()
                queue.extend(staged)
                staged.clear()
                drain_due(2 * qb + pair + 0.5)
                last = qb == NQB - 1 and pair == 1
                if last:
                    # h-major drain: head 0 finishes first so its normalize
                    # overlaps head 1's remaining matmuls
                    tail_kbs = list(lagged)
                    lagged.clear()
                else:
                    while lagged:
                        attnv(*lagged.popleft())
                while epi_q:
                    epi_q.popleft()()

                # normalize off the accumulators: batched reciprocal of the
                # ones-column denominators, then fused mult-evacuate to bf16.
                # Deferred into the next pair's kb==1 so the DVE chain never
                # sits at the PE queue head during the pair transition.
                attnq = pool_sm.tile([128, 4, 2, HD], BF16, tag="attnq")
                rec = pool_sm.tile([128, 2, 4], F32, tag="rec")

                def norm_h(h, accv=accv, attnq=attnq, rec=rec):
                    nc.vector.reciprocal(
                        out=rec[:, h, :],
                        in_=accv[h][:, :, HD : HD + 1].rearrange("p a c -> p (a c)"),
                    )
                    nc.vector.tensor_tensor(
                        out=attnq[:, :, h, :],
                        in0=accv[h][:, :, 0:HD],
                        in1=rec[:, h, :].broadcast_to([128, 4, HD]),
                        op=MUL,
                    )

                def norm_emit():
                    norm_h(0)
                    norm_h(1)

                ep = epilogue_tail(attnq, pair, qb)
                if last:
                    for h in range(2):
                        for expt_, kb_, r_ in tail_kbs:
                            for qt in range(max(r_, 0), 4):
                                nc.tensor.matmul(
                                    accv[h][:, qt, :],
                                    expt_[:, h, qt * 128 : qt * 128 + 128],
                                    vaug_sb[:, kb_, 2 * pair + h, :],
                                    start=False,
                                    stop=(kb_ == 4 * qb + qt),
                                    skip_group_check=True,
                                )
                        norm_h(h)
                    ep()
                else:
                    norm_q.append(norm_emit)
                    epi_q.append(ep)

            if qb == NQB - 1:
                for qs in range(4):
                    reserve.append(oproj_tail(qb * 512 + qs * 128))
            future_oproj -= 4 * 900

        attn_ctx.close()
        pp_tail = ctx.enter_context(
            tc.tile_pool(name="pp_tail", bufs=4, space="PSUM")
        )
        oproj_pool[0] = pp_tail
        while reserve:
            reserve.popleft()()
        while queue:
            _, _, emit = queue.popleft()
            emit()

    nc.compile()
    return nc


_NC = None


def _get_nc():
    global _NC
    if _NC is None:
        _NC = _build()
    return _NC


def kernel(x, mask, Wqkv, bqkv, Wout, bout):
    x = np.asarray(x, dtype=np.float32)
    Wqkv = np.asarray(Wqkv, dtype=np.float32)
    bqkv = np.asarray(bqkv, dtype=np.float32)
    Wout = np.asarray(Wout, dtype=np.float32)
    bout = np.asarray(bout, dtype=np.float32)
    assert not np.any(bqkv), "nonzero bqkv not supported by this kernel"

    import ml_dtypes

    bf16 = ml_dtypes.bfloat16
    f8 = ml_dtypes.float8_e4m3

    def hilo(a):
        hi = a.astype(f8)
        lo = (a - hi.astype(np.float32)).astype(f8)
        return np.ascontiguousarray(hi), np.ascontiguousarray(lo)

    # host-side layout prep; x and the qkv weights ship as fp8 hi/lo pairs
    xhis, xlos = [], []
    for b in range(B):
        xt = x[b].T.reshape(NDT, 128, S).transpose(1, 0, 2)  # [128, 8, 2048]
        hi, lo = hilo(xt)
        xhis.append(hi)
        xlos.append(lo)
    tri = np.triu(np.ones((128, 128), dtype=np.float32)).astype(bf16)
    tri2 = np.ascontiguousarray(np.stack([tri, tri], axis=1))  # [128, 2, 128]
    identv = np.ascontiguousarray(np.eye(128, dtype=np.float32).astype(bf16))

    def wslice(j, g):  # j: 0=q,1=k,2=v -> hi/lo [128, 8, 256] fp8
        cols = Wqkv[:, j * D + g * FL : j * D + (g + 1) * FL] * WSCALE
        wt = cols.reshape(NDT, 128, FL).transpose(1, 0, 2)
        return hilo(wt)

    in_maps = []
    for c in range(8):
        b, g = c // G, c % G
        whq_, wlq_ = wslice(0, g)
        whk_, wlk_ = wslice(1, g)
        whv_, wlv_ = wslice(2, g)
        wo = Wout[g * FL : (g + 1) * FL, :]  # [256, 1024]
        in_maps.append(
            {
                "xhi": xhis[b],
                "xlo": xlos[b],
                "whq": whq_,
                "wlq": wlq_,
                "whk": whk_,
                "wlk": wlk_,
                "whv": whv_,
                "wlv": wlv_,
                "wout": np.ascontiguousarray(
                    wo.reshape(2, 128, D).transpose(1, 0, 2).astype(bf16)
                ),
                "tri2": tri2,
                "ident": identv,
            }
        )

    nc = _get_nc()
    # axon terminals occasionally flake: transient NRT_EXEC_UNIT errors
    # (caught+retried) but also rare silent numeric corruption on a core.
    # Dispatch twice and cross-check; on mismatch, a third run breaks the
    # tie (device execution is deterministic, so good runs agree exactly).
    import time as _time

    def dispatch():
        for attempt in range(3):
            try:
                res = run_bass_kernel_spmd(nc, in_maps, core_ids=list(range(8)))
                break
            except Exception:
                if attempt == 2:
                    raise
                _time.sleep(2.0)
        out = np.empty((B, S, D), dtype=np.float32)
        for b in range(B):
            acc = res.results[b * G]["out_p"].astype(np.float32).copy()
            for g in range(1, G):
                acc += res.results[b * G + g]["out_p"]
            out[b] = acc + bout[None, :]
        return out

    def close(a, b):
        return np.linalg.norm(a - b) <= 1e-4 * np.linalg.norm(a)

    out1 = dispatch()
    out2 = dispatch()
    if close(out1, out2):
        return out1
    out3 = dispatch()
    if close(out1, out3):
        return out1
    if close(out2, out3):
        return out2
    return out3
